# revision 29
# baseline (speedup 1.0000x reference)
"""DiffNet++ (GATv2 diffusion + gamma gating + dot-product prediction) on 8
Trainium2 NeuronCores via Bass/Tile.

Strategy (dst-range edge sharding, one SPMD program):
  - Users/items row-sharded equally: users 98 tiles (12544 rows)/core, items 49
    tiles (6272 rows)/core. Each GAT edge belongs to the core owning its dst.
  - Per core, edges are grouped into blocks of dst "windows" (128 rows each)
    and packed contiguously per (block, src-bank) segment (dma_gather int16
    index => 32768-row banks). Segment capacity is the max over cores; each
    core's tail is -1 indices skipped at runtime via num_idxs_reg (loaded
    per-call from a counts input), so padding costs no DMA descriptors.
  - Segment softmax without max subtraction (logits ~1e-4): out[v] =
    (sum_e exp(e) fs[src]) / (sum_e exp(e)), accumulated via one-hot matmuls
    into PSUM windows; per-dst divide afterwards. dl offsets are
    block-relative; one-hot masks use per-window shifted iota constants.
  - fs[src] rows: dma_gather (batched indirect DMA). fd[dst]: expanded from
    the contiguous dst windows by one-hot fp16 matmuls.
  - Projections row-sharded + AllGather; updated embeddings AllGather/layer.
  - Prediction: edges grouped by dst(item) windows like a GAT: gather hu[src]
    rows only (fp16, 512B); hi[dst] expanded from contiguous hi windows by
    one-hot matmuls; fused multiply-reduce dots; host unpermutes via slotmap.
"""
import sys

sys.path.insert(0, "/opt/trn_rl_repo")

from contextlib import ExitStack

import numpy as np
import ml_dtypes

import concourse.bass as bass
import concourse.tile as tile
from concourse import bacc, mybir
from concourse.bass_utils import run_bass_kernel_spmd
from concourse.masks import make_identity

N_CORES = 8
P = 128
BANK = 32768
GAT_SLOPE = 0.2
MLP_SLOPE = 0.01
F16 = mybir.dt.float16
F32 = mybir.dt.float32
I16 = mybir.dt.int16
I32 = mybir.dt.int32
NPF16 = np.dtype("float16")

Alu = mybir.AluOpType
Act = mybir.ActivationFunctionType


def _ceil(a, b):
    return -(-a // b)


# ---------------------------------------------------------------------------
# host-side preprocessing
# ---------------------------------------------------------------------------

class GatStruct:
    """Packed core-uniform structure for one gather stream's edges.

    Edges are sharded by dst range (S rows/core), grouped into blocks of WB
    windows, and packed contiguously per (block, src-bank) segment sorted by
    window. Segment capacity = ceil(max-core count / 128) subtiles; per-core
    valid counts are shipped separately so trailing -1 slots are skipped by
    the gather at runtime.
    """

    def __init__(self, name, src, dst, table_rows, shard_tiles, wb_cap=72,
                 want_slotmap=False):
        self.name = name
        self.nb = _ceil(table_rows, BANK)
        self.shard_tiles = shard_tiles
        S = shard_tiles * P
        self.S = S

        core = np.minimum(dst // S, N_CORES - 1)
        win = (dst - core * S) // P
        bank = src // BANK

        # WB from the old per-window padding formula (conservative SBUF fit)
        cnt_w = np.zeros((N_CORES, shard_tiles, self.nb), dtype=np.int64)
        np.add.at(cnt_w, (core, win, bank), 1)
        K_win = sum(max(1, int(_ceil(int(cnt_w[:, :, b].max()), P)))
                    for b in range(self.nb))
        self.WB = max(1, min(7, wb_cap // K_win))
        self.blocks = []
        t = shard_tiles
        while t > 0:
            wbi = min(self.WB, t)
            self.blocks.append(wbi)
            t -= wbi
        nblk = len(self.blocks)
        w_base = np.concatenate([[0], np.cumsum(self.blocks)[:-1]])
        self.w_base = w_base

        blk_of_win = np.zeros((shard_tiles,), dtype=np.int64)
        for bi, wbi in enumerate(self.blocks):
            blk_of_win[w_base[bi]:w_base[bi] + wbi] = bi
        blk = blk_of_win[win]

        # per (core, blk, bank) counts
        cnt = np.zeros((N_CORES, nblk, self.nb), dtype=np.int64)
        np.add.at(cnt, (core, blk, bank), 1)
        cnt_max = np.maximum(cnt.max(axis=0), 1)
        self.seg = _ceil(cnt_max, P)            # [nblk, nb] subtiles per seg
        self.G_blk = self.seg.sum(axis=1)       # [nblk]
        self.G_total = int(self.G_blk.sum())
        self.G_max = int(self.G_blk.max())
        self.total_cols = self.G_total * P // 16
        self.ncalls = nblk * self.nb

        order = np.lexsort((win, bank, blk, core))
        src_s = src[order]
        dst_s = dst[order]
        core_s = core[order]
        blk_s = blk[order]
        bank_s = bank[order]
        win_s = win[order]

        self.idx16 = []
        self.dlc = []
        self.dlr = []
        self.counts = []
        self.slotmap = [] if want_slotmap else None
        # per (blk, global-subtile) window ranges (union over cores)
        sub_wlo = np.full((self.G_total,), 10 ** 9, dtype=np.int64)
        sub_whi = np.full((self.G_total,), -1, dtype=np.int64)
        # subtile->block map and block-local fsg column start
        self.sub_of_blk = []
        for bi in range(nblk):
            self.sub_of_blk += [bi] * int(self.G_blk[bi])

        for c in range(N_CORES):
            sel = core_s == c
            csrc = src_s[sel]
            cdst = dst_s[sel]
            cblk = blk_s[sel]
            cbank = bank_s[sel]
            key = cblk * self.nb + cbank
            ids = np.zeros((self.G_total * P,), dtype=np.int16)
            dl = np.full((self.G_total * P,), -1.0, dtype=NPF16)
            smap = (np.full((self.G_total * P,), -1, dtype=np.int64)
                    if want_slotmap else None)
            cn = np.zeros((self.ncalls,), dtype=np.int32)
            eorder = order[sel]
            slot0 = 0
            gsub = 0
            for bi in range(nblk):
                for b in range(self.nb):
                    e0 = np.searchsorted(key, bi * self.nb + b, "left")
                    e1 = np.searchsorted(key, bi * self.nb + b, "right")
                    n = e1 - e0
                    cap = int(self.seg[bi, b]) * P
                    assert n <= cap, (name, c, bi, b, n, cap)
                    ids[slot0:slot0 + n] = (csrc[e0:e1] - b * BANK).astype(np.int16)
                    dl[slot0:slot0 + n] = (
                        cdst[e0:e1] - (c * S + self.w_base[bi] * P)
                    ).astype(NPF16)
                    if want_slotmap:
                        smap[slot0:slot0 + n] = eorder[e0:e1]
                    cn[bi * self.nb + b] = max(n, 1)
                    # window ranges per subtile (only over valid slots)
                    wloc = (cdst[e0:e1] - (c * S + self.w_base[bi] * P)) // P
                    for s in range(_ceil(n, P)):
                        lo = s * P
                        seg_v = wloc[lo:min(lo + P, len(wloc))]
                        if len(seg_v):
                            gg = gsub + s
                            sub_wlo[gg] = min(sub_wlo[gg], int(seg_v.min()))
                            sub_whi[gg] = max(sub_whi[gg], int(seg_v.max()))
                    slot0 += cap
                    gsub += int(self.seg[bi, b])
            assert slot0 == self.G_total * P
            cols = self.total_cols
            a = np.empty((16, cols), dtype=np.int16)
            j = np.arange(self.G_total * P)
            a[j % 16, j // 16] = ids
            self.idx16.append(np.tile(a, (8, 1)))
            self.dlc.append(np.ascontiguousarray(
                dl.reshape(self.G_total, P).T))          # [128, G_total]
            self.dlr.append(dl.reshape(1, -1).copy())    # [1, G_total*128]
            self.counts.append(cn.reshape(1, -1))
            if want_slotmap:
                self.slotmap.append(smap)
        sub_wlo[sub_whi < 0] = 0
        sub_whi[sub_whi < 0] = 0
        self.sub_wlo = sub_wlo
        self.sub_whi = sub_whi


# ---------------------------------------------------------------------------
# program builder
# ---------------------------------------------------------------------------

def bench_pjrt(nc, in_maps, iters=3):
    """Time steady-state executions of the compiled program on the 8 cores."""
    import time as _time
    import jax
    from jax.sharding import Mesh, PartitionSpec
    from jax.experimental.shard_map import shard_map
    from concourse import bass2jax
    from concourse import mybir as _mb

    bass2jax.install_neuronx_cc_hook()
    partition_name = (nc.partition_id_tensor.name
                      if nc.partition_id_tensor else None)
    in_names, out_names, out_avals = [], [], []
    for alloc in nc.m.functions[0].allocations:
        if not isinstance(alloc, _mb.MemoryLocationSet):
            continue
        name = alloc.memorylocations[0].name
        if alloc.kind == "ExternalInput":
            if name != partition_name:
                in_names.append(name)
        elif alloc.kind == "ExternalOutput":
            out_names.append(name)
            out_avals.append(jax.core.ShapedArray(
                tuple(alloc.tensor_shape), _mb.dt.np(alloc.dtype)))
    n_params = len(in_names)
    zero_outs = [np.zeros(a.shape, a.dtype) for a in out_avals]
    all_names = in_names + out_names
    if partition_name is not None:
        all_names = all_names + [partition_name]

    def _body(*args):
        operands = list(args)
        if partition_name is not None:
            operands.append(bass2jax.partition_id_tensor())
        return tuple(bass2jax._bass_exec_p.bind(
            *operands, out_avals=tuple(out_avals),
            in_names=tuple(all_names), out_names=tuple(out_names),
            lowering_input_output_aliases=(), sim_require_finite=True,
            sim_require_nnan=True, nc=nc))

    devices = jax.devices()[:N_CORES]
    mesh = Mesh(np.asarray(devices), ("core",))
    nspec = n_params + len(out_names)
    f = jax.jit(shard_map(_body, mesh=mesh,
                          in_specs=(PartitionSpec("core"),) * nspec,
                          out_specs=(PartitionSpec("core"),) * len(out_names),
                          check_rep=False), keep_unused=True)
    from jax.sharding import NamedSharding
    sh = NamedSharding(mesh, PartitionSpec("core"))
    concat_in = [np.concatenate([np.asarray(m[nm]) for m in in_maps], axis=0)
                 for nm in in_names]
    concat_in += [np.concatenate([z] * N_CORES, axis=0) for z in zero_outs]
    dev_in = [jax.device_put(x, sh) for x in concat_in]
    times = []
    for i in range(iters):
        t0 = _time.time()
        outs = f(*dev_in)
        jax.block_until_ready(outs)
        times.append(_time.time() - t0)
    print(f"[bench] iter times: {[f'{t*1e3:.2f}ms' for t in times]}")
    # pipelined: issue PIPE calls back-to-back, block once
    import os as _osb
    PIPE = int(_osb.environ.get("KPIPE", "64"))
    outs = [f(*dev_in) for _ in range(2)]
    jax.block_until_ready(outs)  # warm
    t0 = _time.time()
    outs = [f(*dev_in) for _ in range(PIPE)]
    jax.block_until_ready(outs)
    piped = (_time.time() - t0) / PIPE
    print(f"[bench] pipelined per-iter: {piped*1e3:.2f}ms")
    return min(min(times[1:]) if len(times) > 1 else times[0], piped)


def build_program(hp):
    U, I, D, L = hp["U"], hp["I"], hp["D"], hp["L"]
    UT, IT = hp["UT"], hp["IT"]
    US, IS = UT * P, IT * P
    UPAD, IPAD = US * N_CORES, IS * N_CORES
    rate, rb, tr = hp["rate"], hp["rb"], hp["tr"]
    pred = hp["pred"]
    PD = hp["PD"]

    nc = bacc.Bacc("TRN2", target_bir_lowering=False, debug=False,
                   num_devices=N_CORES)

    def inp(name, shape, dt):
        return nc.dram_tensor(name, list(shape), dt, kind="ExternalInput")

    user_emb = inp("user_emb", [UPAD, D], F32)       # full, padded
    item_emb = inp("item_emb", [IPAD, D], F32)
    u_shard0 = inp("u_shard0", [US, D], F32)         # per-core slice
    it_shard0 = inp("it_shard0", [IS, D], F32)
    wu = inp("wu", [D, L * 4 * D], F32)
    bu = inp("bu", [P, L * 4 * D], F32)
    wi = inp("wi", [D, L * 2 * D], F32)
    bi_ = inp("bi", [P, L * 2 * D], F32)
    a_in = {g.name: inp(f"a_{g.name}", [P, L * D], F32) for g in (rate, rb, tr)}
    w1 = inp("w1", [2 * D, L * 2 * D], F32)
    b1 = inp("b1", [P, L * 2 * D], F32)
    w2 = inp("w2", [P, L * 2 * D], F32)
    b2 = inp("b2", [P, L * 2], F32)
    im_ext_in = inp("im_ext", [P, 8 * P], F16)   # col j of sec w = j + 128w
    ic_ext_in = inp("ic_ext", [P, 8], F16)       # col w = iota + 128w
    ones_r_in = inp("ones_r", [1, P], F16)

    g_in = {}
    for g in (rate, rb, tr, pred):
        g_in[g.name] = {
            "idx": inp(f"{g.name}_idx", list(g.idx16[0].shape), I16),
            "dlc": inp(f"{g.name}_dlc", list(g.dlc[0].shape), F16),
            "dlr": inp(f"{g.name}_dlr", list(g.dlr[0].shape), F16),
            "cnt": inp(f"{g.name}_cnt", list(g.counts[0].shape), I32),
        }

    pred_out = nc.dram_tensor("pred_out", [P, pred.G_total], F32,
                              kind="ExternalOutput")
    import os
    kphase = os.environ.get("KPHASE", "full")
    dbg_spec = hp.get("dbg_spec")  # (name, rows, cols) of tensor to dump
    dbg_out = None
    if dbg_spec is not None:
        dbg_out = nc.dram_tensor("dbg_out", [dbg_spec[1], dbg_spec[2]], F32,
                                 kind="ExternalOutput")

    def internal(name, shape, shared=False, dt=F32):
        return nc.dram_tensor(name, list(shape), dt,
                              addr_space="Shared" if shared else "Local")

    u_tabs = [user_emb]
    it_tabs = [item_emb]
    u_shards = [u_shard0]
    it_shards = [it_shard0]
    fs_tab = {}      # (gat, l) -> full fs table
    fd_shard = {}    # (gat, l) -> local fd shard (fp16)
    for l in range(L):
        for g, rows_in, rows_out in ((rate, US, UPAD), (rb, IS, IPAD), (tr, US, UPAD)):
            ai = internal(f"agin_fs_{g.name}{l}", [rows_in, D])
            ao = internal(f"fs_{g.name}{l}", [rows_out, D], shared=True)
            fs_tab[(g.name, l)] = (ai, ao)
        fd_shard[("rate", l)] = internal(f"fd_rate{l}", [IS, D], dt=F16)
        fd_shard[("rb", l)] = internal(f"fd_rb{l}", [US, D], dt=F16)
        fd_shard[("tr", l)] = internal(f"fd_tr{l}", [US, D], dt=F16)
        u_shards.append(internal(f"agin_u{l + 1}", [US, D]))
        u_tabs.append(internal(f"u{l + 1}", [UPAD, D], shared=True))
        it_shards.append(internal(f"agin_it{l + 1}", [IS, D]))
        it_tabs.append(internal(f"it{l + 1}", [IPAD, D], shared=True))
    q_sh = internal("q_sh", [US, D])
    p_sh = internal("p_sh", [US, D])
    hu_t = internal("hu", [UPAD, PD], dt=F16)
    hi_sh = internal("hi_sh", [IS, PD], dt=F16)   # core-local item shard

    rg = [list(range(N_CORES))]

    with tile.TileContext(nc) as tc, ExitStack() as topctx:
        const = topctx.enter_context(tc.tile_pool(name="const", bufs=1))

        def cload(t, shape, dt):
            s = const.tile(list(shape), dt, tag=f"c_{t.name}")
            nc.sync.dma_start(out=s[:], in_=t.ap()[:, :])
            return s

        im_ext = cload(im_ext_in, [P, 8 * P], F16)
        ic_ext = cload(ic_ext_in, [P, 8], F16)
        onr = cload(ones_r_in, [1, P], F16)
        ident = const.tile([P, P], F32, tag='c_ident')
        make_identity(nc, ident[:])
        wu_sb = cload(wu, [D, L * 4 * D], F32)
        bu_sb = cload(bu, [P, L * 4 * D], F32)
        wi_sb = cload(wi, [D, L * 2 * D], F32)
        bi_sb = cload(bi_, [P, L * 2 * D], F32)
        a_sb = {nm: cload(a_in[nm], [P, L * D], F32) for nm in a_in}
        w1_sb = cload(w1, [2 * D, L * 2 * D], F32)
        b1_sb = cload(b1, [P, L * 2 * D], F32)
        w2_sb = cload(w2, [P, L * 2 * D], F32)
        b2_sb = cload(b2, [P, L * 2], F32)
        cnt_sb = {g.name: cload(g_in[g.name]["cnt"], [1, g.ncalls], I32)
                  for g in (rate, rb, tr, pred)}

        # ------------------------------------------------------------------
        def proj_phase(l):
            """Row-sharded projections + fd shards, then AllGather fs tables."""
            with ExitStack() as ctx:
                sb = ctx.enter_context(tc.tile_pool(name=f"proj{l}", bufs=2))
                ps = ctx.enter_context(
                    tc.tile_pool(name=f"projp{l}", bufs=4, space="PSUM"))
                pst = ctx.enter_context(
                    tc.tile_pool(name=f"projt{l}", bufs=2, space="PSUM"))

                BT = 8  # node tiles per iteration

                def do(shard_tab, n_tiles, w_sb_l, b_sb_l, ncols, outs):
                    # outs: list of (dst_tensor, col_lo, col_hi, fp16)
                    for t0 in range(0, n_tiles, BT):
                        bt = min(BT, n_tiles - t0)
                        src = sb.tile([P, BT * D], F32, tag="psrc")
                        nc.sync.dma_start(
                            out=src[:, :bt * D].rearrange("p (g d) -> p g d", d=D),
                            in_=shard_tab.ap()[t0 * P:(t0 + bt) * P, :]
                            .rearrange("(g p) d -> p g d", p=P))
                        big = sb.tile([P, BT * ncols], F32, tag="pbig")
                        for k in range(bt):
                            tp = pst.tile([D, P], F32, tag="ptp")
                            nc.tensor.transpose(
                                out=tp[:], in_=src[:, k * D:(k + 1) * D],
                                identity=ident[:])
                            uT = sb.tile([D, P], F32, tag="puT")
                            nc.vector.tensor_copy(out=uT[:], in_=tp[:])
                            mm = ps.tile([P, ncols], F32, tag="pmm")
                            nc.tensor.matmul(mm[:], lhsT=uT[:], rhs=w_sb_l,
                                             start=True, stop=True)
                            nc.vector.tensor_tensor(
                                out=big[:, k * ncols:(k + 1) * ncols],
                                in0=mm[:], in1=b_sb_l, op=Alu.add)
                        n16 = sum(1 for o in outs if o[3])
                        if n16:
                            w16 = sum(o[2] - o[1] for o in outs if o[3])
                            cv = sb.tile([P, BT * w16], F16, tag="pcv")
                            co = 0
                            for (dt_, lo, hi, f16) in outs:
                                if not f16:
                                    continue
                                wdt = hi - lo
                                nc.vector.tensor_copy(
                                    out=cv[:, :bt * w16]
                                    .rearrange("p (g d) -> p g d", d=w16)[:, :, co:co + wdt],
                                    in_=big[:, :bt * ncols]
                                    .rearrange("p (g d) -> p g d", d=ncols)[:, :, lo:hi])
                                co += wdt
                        co = 0
                        for (dt_, lo, hi, f16) in outs:
                            wdt = hi - lo
                            if f16:
                                nc.sync.dma_start(
                                    out=dt_.ap()[t0 * P:(t0 + bt) * P, :]
                                    .rearrange("(g p) d -> p g d", p=P),
                                    in_=cv[:, :bt * w16]
                                    .rearrange("p (g d) -> p g d", d=w16)[:, :, co:co + wdt])
                                co += wdt
                            else:
                                nc.sync.dma_start(
                                    out=dt_.ap()[t0 * P:(t0 + bt) * P, :]
                                    .rearrange("(g p) d -> p g d", p=P),
                                    in_=big[:, :bt * ncols]
                                    .rearrange("p (g d) -> p g d", d=ncols)[:, :, lo:hi])

                do(u_shards[l], UT,
                   wu_sb[:, l * 4 * D:(l + 1) * 4 * D], bu_sb[:, l * 4 * D:(l + 1) * 4 * D], 4 * D,
                   [(fs_tab[("rate", l)][0], 0, D, False),
                    (fs_tab[("tr", l)][0], D, 2 * D, False),
                    (fd_shard[("rb", l)], 2 * D, 3 * D, True),
                    (fd_shard[("tr", l)], 3 * D, 4 * D, True)])
                do(it_shards[l], IT,
                   wi_sb[:, l * 2 * D:(l + 1) * 2 * D], bi_sb[:, l * 2 * D:(l + 1) * 2 * D], 2 * D,
                   [(fd_shard[("rate", l)], 0, D, True),
                    (fs_tab[("rb", l)][0], D, 2 * D, False)])

            import os as _os3
            if _os3.environ.get("KNOAG") == "1":
                return
            for gname in ("rate", "rb", "tr"):
                ai, ao = fs_tab[(gname, l)]
                nc.gpsimd.collective_compute(
                    "AllGather", Alu.bypass, replica_groups=rg,
                    ins=[ai.ap()[:, :]], outs=[ao.ap()[:, :]])

        # ------------------------------------------------------------------
        def gat_phase(l, g, fs_table, fd_sh, out_tensor, resid_tab,
                      octx=None, rep_bufs=2, fde_bufs=2):
            """Edge processing for one GAT; writes out_tensor [S, D] f32."""
            import os as _os
            KG = int(_os.environ.get("KG", "9"))
            gi = g_in[g.name]
            a_l = a_sb[g.name][:, l * D:(l + 1) * D]
            table_rows = fs_table.ap().shape[0]
            GM = g.G_max
            with ExitStack() as _own:
                ctx = octx if octx is not None else _own
                sb = ctx.enter_context(tc.tile_pool(name=f"e{g.name}{l}", bufs=2))
                qp = ctx.enter_context(tc.tile_pool(name=f"eq{g.name}{l}", bufs=3))
                ps_rep = ctx.enter_context(
                    tc.tile_pool(name=f"er{g.name}{l}", bufs=rep_bufs,
                                 space="PSUM"))
                ps_fde = ctx.enter_context(
                    tc.tile_pool(name=f"ef{g.name}{l}", bufs=fde_bufs,
                                 space="PSUM"))
                ps_acc = ctx.enter_context(
                    tc.tile_pool(name=f"ea{g.name}{l}", bufs=2, space="PSUM"))

                nb = g.nb
                w_base = 0
                g_base = 0       # global sub-tile counter
                call = 0
                for bi, wbi in enumerate(g.blocks):
                    G = int(g.G_blk[bi])
                    # loads
                    idx_t = sb.tile([P, GM * P // 16], I16, tag="idx")
                    c0 = g_base * P // 16
                    nc.sync.dma_start(
                        out=idx_t[:, :G * P // 16],
                        in_=gi["idx"].ap()[:, c0:c0 + G * P // 16])
                    dlc_t = sb.tile([P, GM], F16, tag="dlc")
                    nc.sync.dma_start(out=dlc_t[:, :G],
                                      in_=gi["dlc"].ap()[:, g_base:g_base + G])
                    dlr_t = sb.tile([1, GM * P], F16, tag="dlr")
                    nc.sync.dma_start(
                        out=dlr_t[:, :G * P],
                        in_=gi["dlr"].ap()[:, g_base * P:(g_base + G) * P])
                    fd_t = sb.tile([P, g.WB * D], F16, tag="fd")
                    nc.sync.dma_start(
                        out=fd_t[:, :wbi * D],
                        in_=fd_sh.ap()[w_base * P:(w_base + wbi) * P, :]
                        .rearrange("(g p) d -> p g d", p=P))
                    if resid_tab is not None:
                        rs_t = sb.tile([P, g.WB * D], F32, tag="rs")
                        nc.sync.dma_start(
                            out=rs_t[:, :wbi * D],
                            in_=resid_tab.ap()[w_base * P:(w_base + wbi) * P, :]
                            .rearrange("(g p) d -> p g d", p=P))

                    fsg = sb.tile([P, GM * D], F32, tag="fsg")
                    # gathers per bank (packed segments)
                    scol = 0
                    sg = 0
                    for b in (range(nb) if KG >= 1 else []):
                        ngb = int(g.seg[bi, b])
                        if ngb == 0:
                            continue
                        nidx = ngb * P
                        hi_row = min(table_rows, (b + 1) * BANK)
                        nc.gpsimd.dma_gather(
                            fsg[:, sg * D:(sg + ngb) * D]
                            .rearrange("p (g d) -> p g d", d=D),
                            fs_table.ap()[b * BANK:hi_row, :],
                            idx_t[:, scol:scol + nidx // 16],
                            nidx, nidx, D, single_packet=False)
                        scol += nidx // 16
                        sg += ngb
                    if KG <= 1:
                        out_t = sb.tile([P, g.WB * D], F32, tag="out")
                        nc.vector.tensor_copy(out=out_t[:, :wbi * D],
                                              in_=fsg[:, :wbi * D])
                        nc.sync.dma_start(
                            out=out_tensor.ap()[w_base * P:(w_base + wbi) * P, :]
                            .rearrange("(g p) d -> p g d", p=P),
                            in_=out_t[:, :wbi * D].rearrange("p (g d) -> p g d", d=D))
                        w_base += wbi
                        g_base += G
                        continue
                    # acc psum for this block
                    acc = ps_acc.tile([P, g.WB * (D + 1)], F32, tag="acc",
                                      space="PSUM")

                    wlo = g.sub_wlo[g_base:g_base + G]
                    whi = g.sub_whi[g_base:g_base + G]

                    # replicate dlr (groups of 4 sub-tiles)
                    reps = []
                    for r0 in range(0, G, 4):
                        rc = min(4, G - r0)
                        rep = ps_rep.tile([P, 4 * P], F32, tag="rep", space="PSUM")
                        nc.tensor.matmul(
                            rep[:, :rc * P], lhsT=onr[:],
                            rhs=dlr_t[:1, r0 * P:(r0 + rc) * P],
                            start=True, stop=True)
                        reps.append(rep)

                    w_big = sb.tile([P, GM * (D + 1)], F16, tag="wbig")
                    # X-stage groups of 8 sub-tiles
                    for x0 in range(0, G, 8):
                        xc = min(8, G - x0)
                        fde = ps_fde.tile([P, 8 * D], F32, tag="fde", space="PSUM")
                        for j in range(xc):
                            gg = x0 + j
                            rep = reps[gg // 4]
                            for wo in range(int(wlo[gg]), int(whi[gg]) + 1):
                                qt = qp.tile([P, P], F16, tag="qt")
                                nc.vector.tensor_tensor(
                                    out=qt[:],
                                    in0=ic_ext[:, wo:wo + 1].to_broadcast([P, P]),
                                    in1=rep[:, (gg % 4) * P:(gg % 4 + 1) * P],
                                    op=Alu.is_equal)
                                nc.tensor.matmul(
                                    fde[:, j * D:(j + 1) * D], lhsT=qt[:],
                                    rhs=fd_t[:, wo * D:(wo + 1) * D],
                                    start=(wo == int(wlo[gg])),
                                    stop=(wo == int(whi[gg])))
                        # x = fs + fde ; leaky ; e ; z
                        x_t = sb.tile([P, 8 * D], F16, tag="x")
                        nc.vector.tensor_tensor(
                            out=x_t[:, :xc * D],
                            in0=fsg[:, x0 * D:(x0 + xc) * D],
                            in1=fde[:, :xc * D], op=Alu.add)
                        xs = sb.tile([P, 8 * D], F16, tag="xs")
                        nc.vector.tensor_scalar_mul(
                            xs[:, :xc * D], x_t[:, :xc * D], GAT_SLOPE)
                        xl = sb.tile([P, 8 * D], F16, tag="xl")
                        nc.vector.tensor_tensor(
                            out=xl[:, :xc * D], in0=x_t[:, :xc * D],
                            in1=xs[:, :xc * D], op=Alu.max)
                        xa = sb.tile([P, 8 * D], F32, tag="xa")
                        nc.vector.tensor_tensor(
                            out=xa[:, :xc * D], in0=xl[:, :xc * D],
                            in1=a_l.rearrange("p (g d) -> p g d", g=1)
                            .to_broadcast([P, xc, D]),
                            op=Alu.mult)
                        e8 = sb.tile([P, 8], F32, tag="e8")
                        nc.vector.reduce_sum(
                            out=e8[:, :xc],
                            in_=xa[:, :xc * D].rearrange("p (g d) -> p g d", d=D),
                            axis=mybir.AxisListType.X)
                        z8 = sb.tile([P, 8], F32, tag="z8")
                        nc.scalar.activation(z8[:, :xc], e8[:, :xc], Act.Exp)
                        nc.vector.tensor_tensor(
                            out=w_big[:, x0 * (D + 1):(x0 + xc) * (D + 1)]
                            .rearrange("p (g d) -> p g d", d=D + 1)[:, :, 0:D],
                            in0=fsg[:, x0 * D:(x0 + xc) * D]
                            .rearrange("p (g d) -> p g d", d=D),
                            in1=z8[:, :xc].rearrange("p (g d) -> p g d", d=1)
                            .to_broadcast([P, xc, D]),
                            op=Alu.mult)
                        nc.vector.tensor_copy(
                            out=w_big[:, x0 * (D + 1):(x0 + xc) * (D + 1)]
                            .rearrange("p (g d) -> p g d", d=D + 1)[:, :, D:D + 1],
                            in_=z8[:, :xc].rearrange("p (g d) -> p g d", d=1))

                    # accumulation: window-major; each sub-tile contributes to
                    # every window in its [wlo, whi] range via shifted masks
                    for wo in range(wbi):
                        subs = [gg for gg in range(G)
                                if int(wlo[gg]) <= wo <= int(whi[gg])]
                        for si, gg in enumerate(subs):
                            q_t = qp.tile([P, P], F16, tag="q")
                            nc.vector.tensor_tensor(
                                out=q_t[:],
                                in0=dlc_t[:, gg:gg + 1].to_broadcast([P, P]),
                                in1=im_ext[:, wo * P:(wo + 1) * P],
                                op=Alu.is_equal)
                            nc.tensor.matmul(
                                acc[:, wo * (D + 1):(wo + 1) * (D + 1)],
                                lhsT=q_t[:],
                                rhs=w_big[:, gg * (D + 1):(gg + 1) * (D + 1)],
                                start=(si == 0),
                                stop=(si == len(subs) - 1))
                    # divide + store
                    out_t = sb.tile([P, g.WB * D], F32, tag="out")
                    for wo in range(wbi):
                        den = sb.tile([P, 1], F32, tag="den")
                        nc.vector.tensor_scalar_max(
                            den[:], acc[:, wo * (D + 1) + D:(wo + 1) * (D + 1)],
                            1e-30)
                        rec = sb.tile([P, 1], F32, tag="rec")
                        nc.vector.reciprocal(rec[:], den[:])
                        if resid_tab is None:
                            nc.vector.tensor_scalar_mul(
                                out_t[:, wo * D:(wo + 1) * D],
                                acc[:, wo * (D + 1):wo * (D + 1) + D],
                                rec[:, :1])
                        else:
                            tmp = sb.tile([P, D], F32, tag="dtmp")
                            nc.vector.tensor_scalar_mul(
                                tmp[:], acc[:, wo * (D + 1):wo * (D + 1) + D],
                                rec[:, :1])
                            nc.vector.tensor_tensor(
                                out=out_t[:, wo * D:(wo + 1) * D],
                                in0=tmp[:], in1=rs_t[:, wo * D:(wo + 1) * D],
                                op=Alu.add)
                    nc.sync.dma_start(
                        out=out_tensor.ap()[w_base * P:(w_base + wbi) * P, :]
                        .rearrange("(g p) d -> p g d", p=P),
                        in_=out_t[:, :wbi * D].rearrange("p (g d) -> p g d", d=D))

                    w_base += wbi
                    g_base += G

        # ------------------------------------------------------------------
        def epilogue_phase(l, octx=None, mm_bufs=2):
            with ExitStack() as _own:
                ctx = octx if octx is not None else _own
                sb = ctx.enter_context(tc.tile_pool(name=f"ep{l}", bufs=2))
                pst = ctx.enter_context(
                    tc.tile_pool(name=f"ept{l}", bufs=2, space="PSUM"))
                psm = ctx.enter_context(
                    tc.tile_pool(name=f"epm{l}", bufs=mm_bufs, space="PSUM"))
                BT = 8
                w1_l = w1_sb[:, l * 2 * D:(l + 1) * 2 * D]
                b1_l = b1_sb[:, l * 2 * D:(l + 1) * 2 * D]
                w2_l = w2_sb[:, l * 2 * D:(l + 1) * 2 * D]
                b2_l = b2_sb[:, l * 2:(l + 1) * 2]
                for t0 in range(0, UT, BT):
                    bt = min(BT, UT - t0)
                    rows = slice(t0 * P, (t0 + bt) * P)
                    ut = sb.tile([P, BT * D], F32, tag="eu")
                    nc.sync.dma_start(
                        out=ut[:, :bt * D].rearrange("p (g d) -> p g d", d=D),
                        in_=u_shards[l].ap()[rows, :].rearrange("(g p) d -> p g d", p=P))
                    pt = sb.tile([P, BT * D], F32, tag="epp")
                    nc.sync.dma_start(
                        out=pt[:, :bt * D].rearrange("p (g d) -> p g d", d=D),
                        in_=p_sh.ap()[rows, :].rearrange("(g p) d -> p g d", p=P))
                    qt_ = sb.tile([P, BT * D], F32, tag="epq")
                    nc.sync.dma_start(
                        out=qt_[:, :bt * D].rearrange("p (g d) -> p g d", d=D),
                        in_=q_sh.ap()[rows, :].rearrange("(g p) d -> p g d", p=P))
                    ot = sb.tile([P, BT * D], F32, tag="eo")
                    # per-k: transposes + matmuls into batched psum rows
                    mm_inf = psm.tile([P, BT * D], F32, tag="emmi")
                    mm_int = psm.tile([P, BT * D], F32, tag="emmj")
                    for k in range(bt):
                        ts = []
                        for srcp in (ut, pt, qt_):
                            tp = pst.tile([D, P], F32, tag="etp")
                            nc.tensor.transpose(
                                out=tp[:], in_=srcp[:, k * D:(k + 1) * D],
                                identity=ident[:])
                            ts.append(tp)
                        ct_inf = sb.tile([2 * D, P], F32, tag="ecti")
                        nc.vector.tensor_copy(out=ct_inf[0:D, :], in_=ts[0][:])
                        nc.vector.tensor_copy(out=ct_inf[D:2 * D, :], in_=ts[1][:])
                        ct_int = sb.tile([2 * D, P], F32, tag="ectj")
                        nc.vector.tensor_copy(out=ct_int[0:D, :], in_=ts[0][:])
                        nc.vector.tensor_copy(out=ct_int[D:2 * D, :], in_=ts[2][:])
                        nc.tensor.matmul(
                            mm_inf[:, k * D:(k + 1) * D], lhsT=ct_inf[:],
                            rhs=w1_l[:, 0:D], start=True, stop=True)
                        nc.tensor.matmul(
                            mm_int[:, k * D:(k + 1) * D], lhsT=ct_int[:],
                            rhs=w1_l[:, D:2 * D], start=True, stop=True)
                    # batched gating math across the bt tiles
                    sv2 = []
                    for mm_b, col in ((mm_inf, 0), (mm_int, 1)):
                        s1 = sb.tile([P, BT * D], F32, tag="es1")
                        nc.vector.tensor_tensor(
                            out=s1[:, :bt * D].rearrange("p (g d) -> p g d", d=D),
                            in0=mm_b[:, :bt * D].rearrange("p (g d) -> p g d", d=D),
                            in1=b1_l[:, col * D:(col + 1) * D]
                            .rearrange("p (g d) -> p g d", g=1)
                            .to_broadcast([P, bt, D]), op=Alu.add)
                        s1s = sb.tile([P, BT * D], F32, tag="es1s")
                        nc.vector.tensor_scalar_mul(
                            s1s[:, :bt * D], s1[:, :bt * D], MLP_SLOPE)
                        s1l = sb.tile([P, BT * D], F32, tag="es1l")
                        nc.vector.tensor_tensor(
                            out=s1l[:, :bt * D], in0=s1[:, :bt * D],
                            in1=s1s[:, :bt * D], op=Alu.max)
                        xw = sb.tile([P, BT * D], F32, tag="exw")
                        nc.vector.tensor_tensor(
                            out=xw[:, :bt * D].rearrange("p (g d) -> p g d", d=D),
                            in0=s1l[:, :bt * D].rearrange("p (g d) -> p g d", d=D),
                            in1=w2_l[:, col * D:(col + 1) * D]
                            .rearrange("p (g d) -> p g d", g=1)
                            .to_broadcast([P, bt, D]), op=Alu.mult)
                        sv0 = sb.tile([P, BT], F32, tag="esv0")
                        nc.vector.reduce_sum(
                            out=sv0[:, :bt],
                            in_=xw[:, :bt * D].rearrange("p (g d) -> p g d", d=D),
                            axis=mybir.AxisListType.X)
                        sv = sb.tile([P, BT], F32, tag="esv")
                        nc.vector.tensor_tensor(
                            out=sv[:, :bt].rearrange("p (g d) -> p g d", d=1),
                            in0=sv0[:, :bt].rearrange("p (g d) -> p g d", d=1),
                            in1=b2_l[:, col:col + 1]
                            .rearrange("p (g d) -> p g d", g=1)
                            .to_broadcast([P, bt, 1]), op=Alu.add)
                        svs = sb.tile([P, BT], F32, tag="esvs")
                        nc.vector.tensor_scalar_mul(
                            svs[:, :bt], sv[:, :bt], MLP_SLOPE)
                        svl = sb.tile([P, BT], F32, tag="esvl")
                        nc.vector.tensor_tensor(
                            out=svl[:, :bt], in0=sv[:, :bt],
                            in1=svs[:, :bt], op=Alu.max)
                        sv2.append(svl)
                    dg = sb.tile([P, BT], F32, tag="edg")
                    nc.vector.tensor_tensor(
                        out=dg[:, :bt], in0=sv2[0][:, :bt], in1=sv2[1][:, :bt],
                        op=Alu.subtract)
                    g0 = sb.tile([P, BT], F32, tag="eg0")
                    nc.scalar.activation(g0[:, :bt], dg[:, :bt], Act.Sigmoid)
                    pq = sb.tile([P, BT * D], F32, tag="epq2")
                    nc.vector.tensor_tensor(
                        out=pq[:, :bt * D], in0=pt[:, :bt * D],
                        in1=qt_[:, :bt * D], op=Alu.subtract)
                    gpq = sb.tile([P, BT * D], F32, tag="egpq")
                    nc.vector.tensor_tensor(
                        out=gpq[:, :bt * D].rearrange("p (g d) -> p g d", d=D),
                        in0=pq[:, :bt * D].rearrange("p (g d) -> p g d", d=D),
                        in1=g0[:, :bt].rearrange("p (g d) -> p g d", d=1)
                        .to_broadcast([P, bt, D]), op=Alu.mult)
                    uq = sb.tile([P, BT * D], F32, tag="euq")
                    nc.vector.tensor_tensor(
                        out=uq[:, :bt * D], in0=ut[:, :bt * D],
                        in1=qt_[:, :bt * D], op=Alu.add)
                    nc.vector.tensor_tensor(
                        out=ot[:, :bt * D], in0=uq[:, :bt * D],
                        in1=gpq[:, :bt * D], op=Alu.add)
                    nc.sync.dma_start(
                        out=u_shards[l + 1].ap()[rows, :]
                        .rearrange("(g p) d -> p g d", p=P),
                        in_=ot[:, :bt * D].rearrange("p (g d) -> p g d", d=D))

        # ------------------------------------------------------------------
        def hu_build_phase():
            """Assemble hu [UPAD, PD] fp16 (full) and hi_sh [IS, PD] (local)."""
            with ExitStack() as ctx:
                sb = ctx.enter_context(tc.tile_pool(name="hub", bufs=2))
                BT = 16
                for tabs, out_tab, n_tiles in ((u_tabs, hu_t, UPAD // P),
                                               (it_shards, hi_sh, IS // P)):
                    for t0 in range(0, n_tiles, BT):
                        bt = min(BT, n_tiles - t0)
                        rows = slice(t0 * P, (t0 + bt) * P)
                        big = sb.tile([P, BT * PD], F16, tag="hbig")
                        nc.vector.memset(big[:], 0)
                        for li, tab in enumerate(tabs):
                            ld = sb.tile([P, BT * D], F32, tag="hld")
                            nc.sync.dma_start(
                                out=ld[:, :bt * D].rearrange("p (g d) -> p g d", d=D),
                                in_=tab.ap()[rows, :]
                                .rearrange("(g p) d -> p g d", p=P))
                            nc.vector.tensor_copy(
                                out=big[:, :bt * PD]
                                .rearrange("p (g d) -> p g d", d=PD)
                                [:, :, li * D:(li + 1) * D],
                                in_=ld[:, :bt * D]
                                .rearrange("p (g d) -> p g d", d=D))
                        nc.sync.dma_start(
                            out=out_tab.ap()[rows, :]
                            .rearrange("(g p) d -> p g d", p=P),
                            in_=big[:, :bt * PD].rearrange("p (g d) -> p g d", d=PD))

        # ------------------------------------------------------------------
        def pred_phase():
            """Pred edges dst(item)-window grouped: gather hu[src], expand
            hi[dst] via one-hot matmuls, fused dot products."""
            g = pred
            gi = g_in[g.name]
            GM = g.G_max
            with ExitStack() as ctx:
                sb = ctx.enter_context(tc.tile_pool(name="pred", bufs=2))
                qp = ctx.enter_context(tc.tile_pool(name="predq", bufs=3))
                ps_rep = ctx.enter_context(
                    tc.tile_pool(name="predr", bufs=2, space="PSUM"))
                ps_hi = ctx.enter_context(
                    tc.tile_pool(name="predh", bufs=2, space="PSUM"))
                nb = g.nb
                w_base = 0
                g_base = 0
                call = 0
                for bi, wbi in enumerate(g.blocks):
                    G = int(g.G_blk[bi])
                    idx_t = sb.tile([P, GM * P // 16], I16, tag="idx")
                    c0 = g_base * P // 16
                    nc.sync.dma_start(
                        out=idx_t[:, :G * P // 16],
                        in_=gi["idx"].ap()[:, c0:c0 + G * P // 16])
                    dlr_t = sb.tile([1, GM * P], F16, tag="dlr")
                    nc.sync.dma_start(
                        out=dlr_t[:, :G * P],
                        in_=gi["dlr"].ap()[:, g_base * P:(g_base + G) * P])
                    hi_win = sb.tile([P, g.WB * PD], F16, tag="hiw")
                    nc.sync.dma_start(
                        out=hi_win[:, :wbi * PD],
                        in_=hi_sh.ap()[w_base * P:(w_base + wbi) * P, :]
                        .rearrange("(g p) d -> p g d", p=P))

                    hu_g = sb.tile([P, GM * PD], F16, tag="hug")
                    scol = 0
                    sg = 0
                    for b in range(nb):
                        ngb = int(g.seg[bi, b])
                        if ngb == 0:
                            continue
                        nidx = ngb * P
                        hi_row = min(hu_t.ap().shape[0], (b + 1) * BANK)
                        nc.gpsimd.dma_gather(
                            hu_g[:, sg * PD:(sg + ngb) * PD]
                            .rearrange("p (g d) -> p g d", d=PD),
                            hu_t.ap()[b * BANK:hi_row, :],
                            idx_t[:, scol:scol + nidx // 16],
                            nidx, nidx, PD, single_packet=False)
                        scol += nidx // 16
                        sg += ngb

                    wlo = g.sub_wlo[g_base:g_base + G]
                    whi = g.sub_whi[g_base:g_base + G]
                    reps = []
                    for r0 in range(0, G, 4):
                        rc = min(4, G - r0)
                        rep = ps_rep.tile([P, 4 * P], F32, tag="rep", space="PSUM")
                        nc.tensor.matmul(
                            rep[:, :rc * P], lhsT=onr[:],
                            rhs=dlr_t[:1, r0 * P:(r0 + rc) * P],
                            start=True, stop=True)
                        reps.append(rep)

                    dt_ = sb.tile([P, GM], F32, tag="pdot")
                    for x0 in range(0, G, 4):
                        xc = min(4, G - x0)
                        hi_exp = ps_hi.tile([P, 4 * PD], F32, tag="hie",
                                            space="PSUM")
                        for j in range(xc):
                            gg = x0 + j
                            rep = reps[gg // 4]
                            for wo in range(int(wlo[gg]), int(whi[gg]) + 1):
                                qt = qp.tile([P, P], F16, tag="qt")
                                nc.vector.tensor_tensor(
                                    out=qt[:],
                                    in0=ic_ext[:, wo:wo + 1].to_broadcast([P, P]),
                                    in1=rep[:, (gg % 4) * P:(gg % 4 + 1) * P],
                                    op=Alu.is_equal)
                                nc.tensor.matmul(
                                    hi_exp[:, j * PD:(j + 1) * PD], lhsT=qt[:],
                                    rhs=hi_win[:, wo * PD:(wo + 1) * PD],
                                    start=(wo == int(wlo[gg])),
                                    stop=(wo == int(whi[gg])))
                        prod = sb.tile([P, 4 * PD], F32, tag="pprod")
                        nc.vector.tensor_tensor(
                            out=prod[:, :xc * PD],
                            in0=hu_g[:, x0 * PD:(x0 + xc) * PD],
                            in1=hi_exp[:, :xc * PD], op=Alu.mult)
                        nc.vector.reduce_sum(
                            out=dt_[:, x0:x0 + xc],
                            in_=prod[:, :xc * PD]
                            .rearrange("p (g d) -> p g d", d=PD),
                            axis=mybir.AxisListType.X)
                    nc.sync.dma_start(
                        out=pred_out.ap()[:, g_base:g_base + G],
                        in_=dt_[:, :G])
                    w_base += wbi
                    g_base += G
                    call = (bi + 1) * nb

        # ------------------------------------------------------------------
        phase_order = ["null"]
        for l in range(L):
            phase_order += [f"proj{l}", f"rate{l}", f"rb{l}", f"tr{l}",
                            f"epi{l}", f"ag{l}"]
        phase_order += ["hu", "pred"]

        def run_until():
            import os as _os4
            kpair = _os4.environ.get("KPAIR", "1") == "1"
            if kphase == "null":
                return
            for l in range(L):
                proj_phase(l)
                if kphase == f"proj{l}":
                    return
                if kpair:
                    # [rb || tr] then [rate || epi]: compute of one phase
                    # overlaps the other's gather stream (single DMA queue
                    # is the only hard serial resource)
                    with ExitStack() as s1:
                        gat_phase(l, rb, fs_tab[("rb", l)][1],
                                  fd_shard[("rb", l)], q_sh, None,
                                  octx=s1, rep_bufs=1, fde_bufs=1)
                        gat_phase(l, tr, fs_tab[("tr", l)][1],
                                  fd_shard[("tr", l)], p_sh, None,
                                  octx=s1, rep_bufs=1, fde_bufs=1)
                    if kphase == f"tr{l}":
                        return
                    with ExitStack() as s2:
                        gat_phase(l, rate, fs_tab[("rate", l)][1],
                                  fd_shard[("rate", l)], it_shards[l + 1],
                                  it_shards[l],
                                  octx=s2, rep_bufs=1, fde_bufs=1)
                        epilogue_phase(l, octx=s2, mm_bufs=1)
                    if kphase == f"epi{l}":
                        return
                else:
                    gat_phase(l, rate, fs_tab[("rate", l)][1],
                              fd_shard[("rate", l)], it_shards[l + 1],
                              it_shards[l])
                    if kphase == f"rate{l}":
                        return
                    gat_phase(l, rb, fs_tab[("rb", l)][1],
                              fd_shard[("rb", l)], q_sh, None)
                    gat_phase(l, tr, fs_tab[("tr", l)][1],
                              fd_shard[("tr", l)], p_sh, None)
                    if kphase == f"tr{l}":
                        return
                    epilogue_phase(l)
                    if kphase == f"epi{l}":
                        return
                if _os4.environ.get("KNOAG") != "1":
                    for ai, ao in ((u_shards[l + 1], u_tabs[l + 1]),
                                   (it_shards[l + 1], it_tabs[l + 1])):
                        nc.gpsimd.collective_compute(
                            "AllGather", Alu.bypass, replica_groups=rg,
                            ins=[ai.ap()[:, :]], outs=[ao.ap()[:, :]])
                if kphase == f"ag{l}":
                    return
            hu_build_phase()
            if kphase == "hu":
                return
            pred_phase()

        run_until()
        if dbg_out is not None:
            dbg_tensors = dict(
                q_sh=q_sh, p_sh=p_sh, hu=hu_t, hi_sh=hi_sh,
                **{f"u_shard{i}": t for i, t in enumerate(u_shards)},
                **{f"it_shard{i}": t for i, t in enumerate(it_shards)},
                **{f"u_tab{i}": t for i, t in enumerate(u_tabs)},
                **{f"it_tab{i}": t for i, t in enumerate(it_tabs)},
                **{f"fs_{nm}{l}": fs_tab[(nm, l)][1] for nm in ("rate", "rb", "tr")
                   for l in range(L)},
                **{f"fsin_{nm}{l}": fs_tab[(nm, l)][0] for nm in ("rate", "rb", "tr")
                   for l in range(L)},
                **{f"fd_{nm}{l}": fd_shard[(nm, l)] for nm in ("rate", "rb", "tr")
                   for l in range(L)},
            )
            src_t = dbg_tensors[dbg_spec[0]]
            sdt = src_t.ap().dtype
            with ExitStack() as ctx:
                sbd = ctx.enter_context(tc.tile_pool(name="dbg", bufs=2))
                rows, cols = dbg_spec[1], dbg_spec[2]
                for r0 in range(0, rows, P):
                    rc = min(P, rows - r0)
                    t_ = sbd.tile([P, cols], sdt, tag="dbg")
                    nc.sync.dma_start(out=t_[:rc, :],
                                      in_=src_t.ap()[r0:r0 + rc, :])
                    if sdt != F32:
                        t2 = sbd.tile([P, cols], F32, tag="dbg2")
                        nc.vector.tensor_copy(out=t2[:rc, :], in_=t_[:rc, :])
                        t_ = t2
                    nc.sync.dma_start(out=dbg_out.ap()[r0:r0 + rc, :],
                                      in_=t_[:rc, :])

    nc.compile()
    return nc


# ---------------------------------------------------------------------------
# entry point
# ---------------------------------------------------------------------------

def _pad_rows(a, rows):
    out = np.zeros((rows, a.shape[1]), dtype=a.dtype)
    out[:a.shape[0]] = a
    return out


def prepare(inputs):
    """Host preprocessing: returns (hp, in_maps, pred)."""
    U, D = inputs["user_emb"].shape
    I = inputs["item_emb"].shape[0]
    L = inputs["rate_Ws"].shape[0]
    UT = _ceil(_ceil(U, P), N_CORES)
    IT = _ceil(_ceil(I, P), N_CORES)
    US, IS = UT * P, IT * P
    UPAD, IPAD = US * N_CORES, IS * N_CORES
    # gather elem size must be a multiple of 256 bytes -> PD*2 % 256 == 0
    PD = _ceil(D * (L + 1) * 2, 256) * 128

    rate_src = np.asarray(inputs["rate_src"])
    rate_dst = np.asarray(inputs["rate_dst"])
    trust_src = np.asarray(inputs["trust_src"])
    trust_dst = np.asarray(inputs["trust_dst"])

    rate = GatStruct("rate", rate_src, rate_dst, UPAD, IT, wb_cap=112)
    rb = GatStruct("rb", rate_dst, rate_src, IPAD, UT, wb_cap=72)
    tr = GatStruct("tr", trust_src, trust_dst, UPAD, UT, wb_cap=72)

    pos_src = np.asarray(inputs["pos_src"])
    pos_dst = np.asarray(inputs["pos_dst"])
    neg_src = np.asarray(inputs["neg_src"])
    neg_dst = np.asarray(inputs["neg_dst"])
    psrc = np.concatenate([pos_src, neg_src])
    pdst = np.concatenate([pos_dst, neg_dst])
    pred = GatStruct("pred", psrc, pdst, UPAD, IT, wb_cap=40,
                     want_slotmap=True)

    hp = dict(U=U, I=I, D=D, L=L, UT=UT, IT=IT, PD=PD,
              rate=rate, rb=rb, tr=tr, pred=pred)
    print(f"[kernel] struct: rate G={rate.G_total} WB={rate.WB} "
          f"blocks={len(rate.blocks)}; rb G={rb.G_total} WB={rb.WB}; "
          f"tr G={tr.G_total} WB={tr.WB}; pred G={pred.G_total} WB={pred.WB}")

    f16 = NPF16
    ue_pad = _pad_rows(inputs["user_emb"].astype(np.float32), UPAD)
    ie_pad = _pad_rows(inputs["item_emb"].astype(np.float32), IPAD)
    wu = np.concatenate([
        np.concatenate([inputs["rate_Ws"][l], inputs["tr_Ws"][l],
                        inputs["rb_Wd"][l], inputs["tr_Wd"][l]], axis=1)
        for l in range(L)], axis=1).astype(np.float32)
    bu = np.concatenate([
        np.tile(np.concatenate([inputs["rate_bs"][l], inputs["tr_bs"][l],
                                inputs["rb_bd"][l], inputs["tr_bd"][l]])[None, :],
                (P, 1))
        for l in range(L)], axis=1).astype(np.float32)
    wi = np.concatenate([
        np.concatenate([inputs["rate_Wd"][l], inputs["rb_Ws"][l]], axis=1)
        for l in range(L)], axis=1).astype(np.float32)
    bi_ = np.concatenate([
        np.tile(np.concatenate([inputs["rate_bd"][l], inputs["rb_bs"][l]])[None, :],
                (P, 1))
        for l in range(L)], axis=1).astype(np.float32)
    a_arrs = {}
    for nm in ("rate", "rb", "tr"):
        a_arrs[nm] = np.concatenate([
            np.tile(np.asarray(inputs[f"{nm}_a"][l])[None, :], (P, 1))
            for l in range(L)], axis=1).astype(np.float32)
    w1 = np.concatenate([
        np.concatenate([inputs["inf_W1"][l], inputs["int_W1"][l]], axis=1)
        for l in range(L)], axis=1).astype(np.float32)
    b1 = np.concatenate([
        np.tile(np.concatenate([inputs["inf_b1"][l], inputs["int_b1"][l]])[None, :],
                (P, 1))
        for l in range(L)], axis=1).astype(np.float32)
    w2 = np.concatenate([
        np.tile(np.concatenate([inputs["inf_W2"][l][:, 0],
                                inputs["int_W2"][l][:, 0]])[None, :], (P, 1))
        for l in range(L)], axis=1).astype(np.float32)
    b2 = np.concatenate([
        np.tile(np.array([[inputs["inf_b2"][l][0], inputs["int_b2"][l][0]]],
                         dtype=np.float32), (P, 1))
        for l in range(L)], axis=1).astype(np.float32)
    iota = np.arange(P, dtype=np.float32)
    im_ext = np.concatenate(
        [np.tile(iota[None, :] + 128 * w, (P, 1)) for w in range(8)],
        axis=1).astype(f16)
    ic_ext = np.concatenate(
        [iota[:, None] + 128 * w for w in range(8)], axis=1).astype(f16)
    ones_r = np.ones((1, P), dtype=f16)

    in_maps = []
    for c in range(N_CORES):
        m = {
            "user_emb": ue_pad, "item_emb": ie_pad,
            "u_shard0": ue_pad[c * US:(c + 1) * US],
            "it_shard0": ie_pad[c * IS:(c + 1) * IS],
            "wu": wu, "bu": bu, "wi": wi, "bi": bi_,
            "a_rate": a_arrs["rate"], "a_rb": a_arrs["rb"], "a_tr": a_arrs["tr"],
            "w1": w1, "b1": b1, "w2": w2, "b2": b2,
            "im_ext": im_ext, "ic_ext": ic_ext, "ones_r": ones_r,
        }
        for g in (rate, rb, tr, pred):
            m[f"{g.name}_idx"] = g.idx16[c]
            m[f"{g.name}_dlc"] = g.dlc[c]
            m[f"{g.name}_dlr"] = g.dlr[c]
            m[f"{g.name}_cnt"] = g.counts[c]
        in_maps.append(m)
    return hp, in_maps, pred, psrc


def kernel(**inputs):
    import os
    import time as _t
    hp, in_maps, pred, psrc = prepare(inputs)

    kdbg = os.environ.get("KDBG")
    if kdbg:
        U, D = inputs["user_emb"].shape
        L = inputs["rate_Ws"].shape[0]
        UT, IT = hp["UT"], hp["IT"]
        US, IS = UT * P, IT * P
        UPAD, IPAD = US * N_CORES, IS * N_CORES
        PD = hp["PD"]
        shp = {}
        for i in range(L + 1):
            shp[f"u_shard{i}"] = (US, D); shp[f"it_shard{i}"] = (IS, D)
            shp[f"u_tab{i}"] = (UPAD, D); shp[f"it_tab{i}"] = (IPAD, D)
        for l in range(L):
            shp[f"fs_rate{l}"] = (UPAD, D); shp[f"fs_tr{l}"] = (UPAD, D)
            shp[f"fs_rb{l}"] = (IPAD, D)
            shp[f"fsin_rate{l}"] = (US, D); shp[f"fsin_tr{l}"] = (US, D)
            shp[f"fsin_rb{l}"] = (IS, D)
            shp[f"fd_rate{l}"] = (IS, D); shp[f"fd_rb{l}"] = (US, D)
            shp[f"fd_tr{l}"] = (US, D)
        shp["q_sh"] = (US, D); shp["p_sh"] = (US, D)
        shp["hu"] = (UPAD, PD); shp["hi"] = (IPAD, PD)
        hp["dbg_spec"] = (kdbg, *shp[kdbg])

    t_b = _t.time()
    nc = build_program(hp)
    print(f"[kernel] build+compile: {_t.time() - t_b:.1f}s")

    trace = os.environ.get("KTRACE") == "1"
    t_run = _t.time()
    res = run_bass_kernel_spmd(nc, in_maps, core_ids=list(range(N_CORES)),
                               trace=trace)
    print(f"[kernel] device run wall: {_t.time() - t_run:.1f}s")
    global LAST_RES, LAST_HP, LAST_EXEC_NS
    LAST_RES, LAST_HP, LAST_EXEC_NS = res, hp, res.exec_time_ns
    if os.environ.get("KBENCH") == "1":
        tmin = bench_pjrt(nc, in_maps, iters=int(os.environ.get("KBENCH_ITERS", "4")))
        LAST_EXEC_NS = int(tmin * 1e9)

    # ---- assemble outputs ----
    E = len(psrc)
    n_pos = len(np.asarray(inputs["pos_src"]))
    out = np.zeros((E,), dtype=np.float32)
    for c in range(N_CORES):
        vals = res.results[c]["pred_out"]  # [128, G_total]
        smap = pred.slotmap[c]
        gidx = np.arange(len(smap))
        v = vals[gidx % P, gidx // P]
        ok = smap >= 0
        out[smap[ok]] = v[ok]
    pos = out[:n_pos].reshape(-1, 1)
    neg = out[n_pos:].reshape(-1, 1)
    return pos, neg


# revision 32
# speedup vs baseline: 1.1441x; 1.1441x over previous
"""DiffNet++ (GATv2 diffusion + gamma gating + dot-product prediction) on 8
Trainium2 NeuronCores via Bass/Tile.

Strategy (dst-range edge sharding, one SPMD program):
  - Users/items row-sharded equally: users 98 tiles (12544 rows)/core, items 49
    tiles (6272 rows)/core. Each GAT edge belongs to the core owning its dst.
  - Per core, edges are grouped into blocks of dst "windows" (128 rows each)
    and packed contiguously per (block, src-bank) segment (dma_gather int16
    index => 32768-row banks). Segment capacity is the max over cores; each
    core's tail is -1 indices skipped at runtime via num_idxs_reg (loaded
    per-call from a counts input), so padding costs no DMA descriptors.
  - Segment softmax without max subtraction (logits ~1e-4): out[v] =
    (sum_e exp(e) fs[src]) / (sum_e exp(e)), accumulated via one-hot matmuls
    into PSUM windows; per-dst divide afterwards. dl offsets are
    block-relative; one-hot masks use per-window shifted iota constants.
  - fs[src] rows: dma_gather (batched indirect DMA). fd[dst]: expanded from
    the contiguous dst windows by one-hot fp16 matmuls.
  - Projections row-sharded + AllGather; updated embeddings AllGather/layer.
  - Prediction: edges grouped by dst(item) windows like a GAT: gather hu[src]
    rows only (fp16, 512B); hi[dst] expanded from contiguous hi windows by
    one-hot matmuls; fused multiply-reduce dots; host unpermutes via slotmap.
"""
import sys

sys.path.insert(0, "/opt/trn_rl_repo")

from contextlib import ExitStack

import numpy as np
import ml_dtypes

import concourse.bass as bass
import concourse.tile as tile
from concourse import bacc, mybir
from concourse.bass_utils import run_bass_kernel_spmd
from concourse.masks import make_identity

N_CORES = 8
P = 128
BANK = 32768
GAT_SLOPE = 0.2
MLP_SLOPE = 0.01
F16 = mybir.dt.float16
F32 = mybir.dt.float32
I16 = mybir.dt.int16
I32 = mybir.dt.int32
NPF16 = np.dtype("float16")

Alu = mybir.AluOpType
Act = mybir.ActivationFunctionType


def _ceil(a, b):
    return -(-a // b)


# ---------------------------------------------------------------------------
# host-side preprocessing
# ---------------------------------------------------------------------------

class GatStruct:
    """Packed core-uniform structure for one gather stream's edges.

    Edges are sharded by dst range (S rows/core), grouped into blocks of WB
    windows, and packed contiguously per (block, src-bank) segment sorted by
    window. Segment capacity = ceil(max-core count / 128) subtiles; per-core
    valid counts are shipped separately so trailing -1 slots are skipped by
    the gather at runtime.
    """

    def __init__(self, name, src, dst, table_rows, shard_tiles, wb_cap=72,
                 want_slotmap=False):
        self.name = name
        self.nb = _ceil(table_rows, BANK)
        self.shard_tiles = shard_tiles
        S = shard_tiles * P
        self.S = S

        core = np.minimum(dst // S, N_CORES - 1)
        win = (dst - core * S) // P
        bank = src // BANK

        # WB from the old per-window padding formula (conservative SBUF fit)
        cnt_w = np.zeros((N_CORES, shard_tiles, self.nb), dtype=np.int64)
        np.add.at(cnt_w, (core, win, bank), 1)
        K_win = sum(max(1, int(_ceil(int(cnt_w[:, :, b].max()), P)))
                    for b in range(self.nb))
        self.WB = max(1, min(7, wb_cap // K_win))
        self.blocks = []
        t = shard_tiles
        while t > 0:
            wbi = min(self.WB, t)
            self.blocks.append(wbi)
            t -= wbi
        nblk = len(self.blocks)
        w_base = np.concatenate([[0], np.cumsum(self.blocks)[:-1]])
        self.w_base = w_base

        blk_of_win = np.zeros((shard_tiles,), dtype=np.int64)
        for bi, wbi in enumerate(self.blocks):
            blk_of_win[w_base[bi]:w_base[bi] + wbi] = bi
        blk = blk_of_win[win]

        # per (core, blk, bank) counts
        cnt = np.zeros((N_CORES, nblk, self.nb), dtype=np.int64)
        np.add.at(cnt, (core, blk, bank), 1)
        cnt_max = np.maximum(cnt.max(axis=0), 1)
        self.seg = _ceil(cnt_max, P)            # [nblk, nb] subtiles per seg
        self.G_blk = self.seg.sum(axis=1)       # [nblk]
        self.G_total = int(self.G_blk.sum())
        self.G_max = int(self.G_blk.max())
        self.total_cols = self.G_total * P // 16
        self.ncalls = nblk * self.nb

        order = np.lexsort((win, bank, blk, core))
        src_s = src[order]
        dst_s = dst[order]
        core_s = core[order]
        blk_s = blk[order]
        bank_s = bank[order]
        win_s = win[order]

        self.idx16 = []
        self.dlc = []
        self.dlr = []
        self.counts = []
        self.slotmap = [] if want_slotmap else None
        # per (blk, global-subtile) window ranges (union over cores)
        sub_wlo = np.full((self.G_total,), 10 ** 9, dtype=np.int64)
        sub_whi = np.full((self.G_total,), -1, dtype=np.int64)
        # subtile->block map and block-local fsg column start
        self.sub_of_blk = []
        for bi in range(nblk):
            self.sub_of_blk += [bi] * int(self.G_blk[bi])

        for c in range(N_CORES):
            sel = core_s == c
            csrc = src_s[sel]
            cdst = dst_s[sel]
            cblk = blk_s[sel]
            cbank = bank_s[sel]
            key = cblk * self.nb + cbank
            ids = np.zeros((self.G_total * P,), dtype=np.int16)
            dl = np.full((self.G_total * P,), -1.0, dtype=NPF16)
            smap = (np.full((self.G_total * P,), -1, dtype=np.int64)
                    if want_slotmap else None)
            cn = np.zeros((self.ncalls,), dtype=np.int32)
            eorder = order[sel]
            slot0 = 0
            gsub = 0
            for bi in range(nblk):
                for b in range(self.nb):
                    e0 = np.searchsorted(key, bi * self.nb + b, "left")
                    e1 = np.searchsorted(key, bi * self.nb + b, "right")
                    n = e1 - e0
                    cap = int(self.seg[bi, b]) * P
                    assert n <= cap, (name, c, bi, b, n, cap)
                    ids[slot0:slot0 + n] = (csrc[e0:e1] - b * BANK).astype(np.int16)
                    dl[slot0:slot0 + n] = (
                        cdst[e0:e1] - (c * S + self.w_base[bi] * P)
                    ).astype(NPF16)
                    if want_slotmap:
                        smap[slot0:slot0 + n] = eorder[e0:e1]
                    cn[bi * self.nb + b] = max(n, 1)
                    # window ranges per subtile (only over valid slots)
                    wloc = (cdst[e0:e1] - (c * S + self.w_base[bi] * P)) // P
                    for s in range(_ceil(n, P)):
                        lo = s * P
                        seg_v = wloc[lo:min(lo + P, len(wloc))]
                        if len(seg_v):
                            gg = gsub + s
                            sub_wlo[gg] = min(sub_wlo[gg], int(seg_v.min()))
                            sub_whi[gg] = max(sub_whi[gg], int(seg_v.max()))
                    slot0 += cap
                    gsub += int(self.seg[bi, b])
            assert slot0 == self.G_total * P
            cols = self.total_cols
            a = np.empty((16, cols), dtype=np.int16)
            j = np.arange(self.G_total * P)
            a[j % 16, j // 16] = ids
            self.idx16.append(np.tile(a, (8, 1)))
            self.dlc.append(np.ascontiguousarray(
                dl.reshape(self.G_total, P).T))          # [128, G_total]
            self.dlr.append(dl.reshape(1, -1).copy())    # [1, G_total*128]
            self.counts.append(cn.reshape(1, -1))
            if want_slotmap:
                self.slotmap.append(smap)
        sub_wlo[sub_whi < 0] = 0
        sub_whi[sub_whi < 0] = 0
        self.sub_wlo = sub_wlo
        self.sub_whi = sub_whi


# ---------------------------------------------------------------------------
# program builder
# ---------------------------------------------------------------------------

def bench_pjrt(nc, in_maps, iters=3):
    """Time steady-state executions of the compiled program on the 8 cores."""
    import time as _time
    import jax
    from jax.sharding import Mesh, PartitionSpec
    from jax.experimental.shard_map import shard_map
    from concourse import bass2jax
    from concourse import mybir as _mb

    bass2jax.install_neuronx_cc_hook()
    partition_name = (nc.partition_id_tensor.name
                      if nc.partition_id_tensor else None)
    in_names, out_names, out_avals = [], [], []
    for alloc in nc.m.functions[0].allocations:
        if not isinstance(alloc, _mb.MemoryLocationSet):
            continue
        name = alloc.memorylocations[0].name
        if alloc.kind == "ExternalInput":
            if name != partition_name:
                in_names.append(name)
        elif alloc.kind == "ExternalOutput":
            out_names.append(name)
            out_avals.append(jax.core.ShapedArray(
                tuple(alloc.tensor_shape), _mb.dt.np(alloc.dtype)))
    n_params = len(in_names)
    zero_outs = [np.zeros(a.shape, a.dtype) for a in out_avals]
    all_names = in_names + out_names
    if partition_name is not None:
        all_names = all_names + [partition_name]

    def _body(*args):
        operands = list(args)
        if partition_name is not None:
            operands.append(bass2jax.partition_id_tensor())
        return tuple(bass2jax._bass_exec_p.bind(
            *operands, out_avals=tuple(out_avals),
            in_names=tuple(all_names), out_names=tuple(out_names),
            lowering_input_output_aliases=(), sim_require_finite=True,
            sim_require_nnan=True, nc=nc))

    devices = jax.devices()[:N_CORES]
    mesh = Mesh(np.asarray(devices), ("core",))
    nspec = n_params + len(out_names)
    f = jax.jit(shard_map(_body, mesh=mesh,
                          in_specs=(PartitionSpec("core"),) * nspec,
                          out_specs=(PartitionSpec("core"),) * len(out_names),
                          check_rep=False), keep_unused=True)
    from jax.sharding import NamedSharding
    sh = NamedSharding(mesh, PartitionSpec("core"))
    concat_in = [np.concatenate([np.asarray(m[nm]) for m in in_maps], axis=0)
                 for nm in in_names]
    concat_in += [np.concatenate([z] * N_CORES, axis=0) for z in zero_outs]
    dev_in = [jax.device_put(x, sh) for x in concat_in]
    times = []
    for i in range(iters):
        t0 = _time.time()
        outs = f(*dev_in)
        jax.block_until_ready(outs)
        times.append(_time.time() - t0)
    print(f"[bench] iter times: {[f'{t*1e3:.2f}ms' for t in times]}")
    # pipelined: issue PIPE calls back-to-back, block once
    import os as _osb
    PIPE = int(_osb.environ.get("KPIPE", "64"))
    outs = [f(*dev_in) for _ in range(2)]
    jax.block_until_ready(outs)  # warm
    t0 = _time.time()
    outs = [f(*dev_in) for _ in range(PIPE)]
    jax.block_until_ready(outs)
    piped = (_time.time() - t0) / PIPE
    print(f"[bench] pipelined per-iter: {piped*1e3:.2f}ms")
    return min(min(times[1:]) if len(times) > 1 else times[0], piped)


def build_program(hp):
    U, I, D, L = hp["U"], hp["I"], hp["D"], hp["L"]
    UT, IT = hp["UT"], hp["IT"]
    US, IS = UT * P, IT * P
    UPAD, IPAD = US * N_CORES, IS * N_CORES
    rate, rb, tr = hp["rate"], hp["rb"], hp["tr"]
    pred = hp["pred"]
    PD = hp["PD"]

    nc = bacc.Bacc("TRN2", target_bir_lowering=False, debug=False,
                   num_devices=N_CORES)

    def inp(name, shape, dt):
        return nc.dram_tensor(name, list(shape), dt, kind="ExternalInput")

    user_emb = inp("user_emb", [UPAD, D], F32)       # full, padded
    item_emb = inp("item_emb", [IPAD, D], F32)
    u_shard0 = inp("u_shard0", [US, D], F32)         # per-core slice
    it_shard0 = inp("it_shard0", [IS, D], F32)
    # all small weights packed into one f32 blob [P, 2692] (col layout below)
    # and f16 constants into one blob [P, 1160]; cuts PJRT buffer count
    WCOLS = 2692
    FCOLS = 1160
    wblob_in = inp("wblob", [P, WCOLS], F32)
    fblob_in = inp("fblob", [P, FCOLS], F16)

    g_in = {}
    for g in (rate, rb, tr, pred):
        g_in[g.name] = {
            "idx": inp(f"{g.name}_idx", list(g.idx16[0].shape), I16),
            "dlc": inp(f"{g.name}_dlc", list(g.dlc[0].shape), F16),
            "dlr": inp(f"{g.name}_dlr", list(g.dlr[0].shape), F16),
        }

    pred_out = nc.dram_tensor("pred_out", [P, pred.G_total], F32,
                              kind="ExternalOutput")
    import os
    kphase = os.environ.get("KPHASE", "full")
    dbg_spec = hp.get("dbg_spec")  # (name, rows, cols) of tensor to dump
    dbg_out = None
    if dbg_spec is not None:
        dbg_out = nc.dram_tensor("dbg_out", [dbg_spec[1], dbg_spec[2]], F32,
                                 kind="ExternalOutput")

    def internal(name, shape, shared=False, dt=F32):
        return nc.dram_tensor(name, list(shape), dt,
                              addr_space="Shared" if shared else "Local")

    u_tabs = [user_emb]
    it_tabs = [item_emb]
    u_shards = [u_shard0]
    it_shards = [it_shard0]
    fs_tab = {}      # (gat, l) -> full fs table
    fd_shard = {}    # (gat, l) -> local fd shard (fp16)
    for l in range(L):
        for g, rows_in, rows_out in ((rate, US, UPAD), (rb, IS, IPAD), (tr, US, UPAD)):
            ai = internal(f"agin_fs_{g.name}{l}", [rows_in, D])
            ao = internal(f"fs_{g.name}{l}", [rows_out, D], shared=True)
            fs_tab[(g.name, l)] = (ai, ao)
        fd_shard[("rate", l)] = internal(f"fd_rate{l}", [IS, D], dt=F16)
        fd_shard[("rb", l)] = internal(f"fd_rb{l}", [US, D], dt=F16)
        fd_shard[("tr", l)] = internal(f"fd_tr{l}", [US, D], dt=F16)
        u_shards.append(internal(f"agin_u{l + 1}", [US, D]))
        u_tabs.append(internal(f"u{l + 1}", [UPAD, D], shared=True))
        it_shards.append(internal(f"agin_it{l + 1}", [IS, D]))
        it_tabs.append(internal(f"it{l + 1}", [IPAD, D], shared=True))
    q_sh = internal("q_sh", [US, D])
    p_sh = internal("p_sh", [US, D])
    hu_t = internal("hu", [UPAD, PD], dt=F16)
    hi_sh = internal("hi_sh", [IS, PD], dt=F16)   # core-local item shard

    rg = [list(range(N_CORES))]

    with tile.TileContext(nc) as tc, ExitStack() as topctx:
        const = topctx.enter_context(tc.tile_pool(name="const", bufs=1))

        def cload(t, shape, dt):
            s = const.tile(list(shape), dt, tag=f"c_{t.name}")
            nc.sync.dma_start(out=s[:], in_=t.ap()[:, :])
            return s

        ident = const.tile([P, P], F32, tag='c_ident')
        make_identity(nc, ident[:])
        wb = cload(wblob_in, [P, WCOLS], F32)
        wu_sb = wb[0:D, 0:512]
        bu_sb = wb[:, 512:1024]
        wi_sb = wb[0:D, 1024:1280]
        bi_sb = wb[:, 1280:1536]
        a_sb = {"rate": wb[:, 1536:1664], "rb": wb[:, 1664:1792],
                "tr": wb[:, 1792:1920]}
        w1_sb = wb[:, 1920:2176]
        b1_sb = wb[:, 2176:2432]
        w2_sb = wb[:, 2432:2688]
        b2_sb = wb[:, 2688:2692]
        fb = cload(fblob_in, [P, FCOLS], F16)
        im_ext = fb[:, 0:8 * P]
        ic_ext = fb[:, 8 * P:8 * P + 8]
        onr = fb[0:1, 8 * P + 8:8 * P + 8 + P]

        # ------------------------------------------------------------------
        def proj_phase(l):
            """Row-sharded projections + fd shards, then AllGather fs tables."""
            with ExitStack() as ctx:
                sb = ctx.enter_context(tc.tile_pool(name=f"proj{l}", bufs=2))
                ps = ctx.enter_context(
                    tc.tile_pool(name=f"projp{l}", bufs=4, space="PSUM"))
                pst = ctx.enter_context(
                    tc.tile_pool(name=f"projt{l}", bufs=2, space="PSUM"))

                BT = 8  # node tiles per iteration

                def do(shard_tab, n_tiles, w_sb_l, b_sb_l, ncols, outs):
                    # outs: list of (dst_tensor, col_lo, col_hi, fp16)
                    for t0 in range(0, n_tiles, BT):
                        bt = min(BT, n_tiles - t0)
                        src = sb.tile([P, BT * D], F32, tag="psrc")
                        nc.sync.dma_start(
                            out=src[:, :bt * D].rearrange("p (g d) -> p g d", d=D),
                            in_=shard_tab.ap()[t0 * P:(t0 + bt) * P, :]
                            .rearrange("(g p) d -> p g d", p=P))
                        big = sb.tile([P, BT * ncols], F32, tag="pbig")
                        for k in range(bt):
                            tp = pst.tile([D, P], F32, tag="ptp")
                            nc.tensor.transpose(
                                out=tp[:], in_=src[:, k * D:(k + 1) * D],
                                identity=ident[:])
                            uT = sb.tile([D, P], F32, tag="puT")
                            nc.vector.tensor_copy(out=uT[:], in_=tp[:])
                            mm = ps.tile([P, ncols], F32, tag="pmm")
                            nc.tensor.matmul(mm[:], lhsT=uT[:], rhs=w_sb_l,
                                             start=True, stop=True)
                            nc.vector.tensor_tensor(
                                out=big[:, k * ncols:(k + 1) * ncols],
                                in0=mm[:], in1=b_sb_l, op=Alu.add)
                        n16 = sum(1 for o in outs if o[3])
                        if n16:
                            w16 = sum(o[2] - o[1] for o in outs if o[3])
                            cv = sb.tile([P, BT * w16], F16, tag="pcv")
                            co = 0
                            for (dt_, lo, hi, f16) in outs:
                                if not f16:
                                    continue
                                wdt = hi - lo
                                nc.vector.tensor_copy(
                                    out=cv[:, :bt * w16]
                                    .rearrange("p (g d) -> p g d", d=w16)[:, :, co:co + wdt],
                                    in_=big[:, :bt * ncols]
                                    .rearrange("p (g d) -> p g d", d=ncols)[:, :, lo:hi])
                                co += wdt
                        co = 0
                        for (dt_, lo, hi, f16) in outs:
                            wdt = hi - lo
                            if f16:
                                nc.sync.dma_start(
                                    out=dt_.ap()[t0 * P:(t0 + bt) * P, :]
                                    .rearrange("(g p) d -> p g d", p=P),
                                    in_=cv[:, :bt * w16]
                                    .rearrange("p (g d) -> p g d", d=w16)[:, :, co:co + wdt])
                                co += wdt
                            else:
                                nc.sync.dma_start(
                                    out=dt_.ap()[t0 * P:(t0 + bt) * P, :]
                                    .rearrange("(g p) d -> p g d", p=P),
                                    in_=big[:, :bt * ncols]
                                    .rearrange("p (g d) -> p g d", d=ncols)[:, :, lo:hi])

                do(u_shards[l], UT,
                   wu_sb[:, l * 4 * D:(l + 1) * 4 * D], bu_sb[:, l * 4 * D:(l + 1) * 4 * D], 4 * D,
                   [(fs_tab[("rate", l)][0], 0, D, False),
                    (fs_tab[("tr", l)][0], D, 2 * D, False),
                    (fd_shard[("rb", l)], 2 * D, 3 * D, True),
                    (fd_shard[("tr", l)], 3 * D, 4 * D, True)])
                do(it_shards[l], IT,
                   wi_sb[:, l * 2 * D:(l + 1) * 2 * D], bi_sb[:, l * 2 * D:(l + 1) * 2 * D], 2 * D,
                   [(fd_shard[("rate", l)], 0, D, True),
                    (fs_tab[("rb", l)][0], D, 2 * D, False)])

            import os as _os3
            if _os3.environ.get("KNOAG") == "1":
                return
            for gname in ("rate", "rb", "tr"):
                ai, ao = fs_tab[(gname, l)]
                nc.gpsimd.collective_compute(
                    "AllGather", Alu.bypass, replica_groups=rg,
                    ins=[ai.ap()[:, :]], outs=[ao.ap()[:, :]])

        # ------------------------------------------------------------------
        def gat_phase(l, g, fs_table, fd_sh, out_tensor, resid_tab,
                      octx=None, rep_bufs=2, fde_bufs=2):
            """Edge processing for one GAT; writes out_tensor [S, D] f32."""
            import os as _os
            KG = int(_os.environ.get("KG", "9"))
            gi = g_in[g.name]
            a_l = a_sb[g.name][:, l * D:(l + 1) * D]
            table_rows = fs_table.ap().shape[0]
            GM = g.G_max
            with ExitStack() as _own:
                ctx = octx if octx is not None else _own
                sb = ctx.enter_context(tc.tile_pool(name=f"e{g.name}{l}", bufs=2))
                qp = ctx.enter_context(tc.tile_pool(name=f"eq{g.name}{l}", bufs=3))
                ps_rep = ctx.enter_context(
                    tc.tile_pool(name=f"er{g.name}{l}", bufs=rep_bufs,
                                 space="PSUM"))
                ps_fde = ctx.enter_context(
                    tc.tile_pool(name=f"ef{g.name}{l}", bufs=fde_bufs,
                                 space="PSUM"))
                ps_acc = ctx.enter_context(
                    tc.tile_pool(name=f"ea{g.name}{l}", bufs=2, space="PSUM"))

                nb = g.nb
                w_base = 0
                g_base = 0       # global sub-tile counter
                call = 0
                for bi, wbi in enumerate(g.blocks):
                    G = int(g.G_blk[bi])
                    # loads
                    idx_t = sb.tile([P, GM * P // 16], I16, tag="idx")
                    c0 = g_base * P // 16
                    nc.sync.dma_start(
                        out=idx_t[:, :G * P // 16],
                        in_=gi["idx"].ap()[:, c0:c0 + G * P // 16])
                    dlc_t = sb.tile([P, GM], F16, tag="dlc")
                    nc.sync.dma_start(out=dlc_t[:, :G],
                                      in_=gi["dlc"].ap()[:, g_base:g_base + G])
                    dlr_t = sb.tile([1, GM * P], F16, tag="dlr")
                    nc.sync.dma_start(
                        out=dlr_t[:, :G * P],
                        in_=gi["dlr"].ap()[:, g_base * P:(g_base + G) * P])
                    fd_t = sb.tile([P, g.WB * D], F16, tag="fd")
                    nc.sync.dma_start(
                        out=fd_t[:, :wbi * D],
                        in_=fd_sh.ap()[w_base * P:(w_base + wbi) * P, :]
                        .rearrange("(g p) d -> p g d", p=P))
                    if resid_tab is not None:
                        rs_t = sb.tile([P, g.WB * D], F32, tag="rs")
                        nc.sync.dma_start(
                            out=rs_t[:, :wbi * D],
                            in_=resid_tab.ap()[w_base * P:(w_base + wbi) * P, :]
                            .rearrange("(g p) d -> p g d", p=P))

                    fsg = sb.tile([P, GM * D], F32, tag="fsg")
                    # gathers per bank (packed segments)
                    scol = 0
                    sg = 0
                    for b in (range(nb) if KG >= 1 else []):
                        ngb = int(g.seg[bi, b])
                        if ngb == 0:
                            continue
                        nidx = ngb * P
                        hi_row = min(table_rows, (b + 1) * BANK)
                        nc.gpsimd.dma_gather(
                            fsg[:, sg * D:(sg + ngb) * D]
                            .rearrange("p (g d) -> p g d", d=D),
                            fs_table.ap()[b * BANK:hi_row, :],
                            idx_t[:, scol:scol + nidx // 16],
                            nidx, nidx, D, single_packet=False)
                        scol += nidx // 16
                        sg += ngb
                    if KG <= 1:
                        out_t = sb.tile([P, g.WB * D], F32, tag="out")
                        nc.vector.tensor_copy(out=out_t[:, :wbi * D],
                                              in_=fsg[:, :wbi * D])
                        nc.sync.dma_start(
                            out=out_tensor.ap()[w_base * P:(w_base + wbi) * P, :]
                            .rearrange("(g p) d -> p g d", p=P),
                            in_=out_t[:, :wbi * D].rearrange("p (g d) -> p g d", d=D))
                        w_base += wbi
                        g_base += G
                        continue
                    # acc psum for this block
                    acc = ps_acc.tile([P, g.WB * (D + 1)], F32, tag="acc",
                                      space="PSUM")

                    wlo = g.sub_wlo[g_base:g_base + G]
                    whi = g.sub_whi[g_base:g_base + G]

                    # replicate dlr (groups of 4 sub-tiles)
                    reps = []
                    for r0 in range(0, G, 4):
                        rc = min(4, G - r0)
                        rep = ps_rep.tile([P, 4 * P], F32, tag="rep", space="PSUM")
                        nc.tensor.matmul(
                            rep[:, :rc * P], lhsT=onr[:],
                            rhs=dlr_t[:1, r0 * P:(r0 + rc) * P],
                            start=True, stop=True)
                        reps.append(rep)

                    w_big = sb.tile([P, GM * (D + 1)], F16, tag="wbig")
                    # X-stage groups of 8 sub-tiles
                    for x0 in range(0, G, 8):
                        xc = min(8, G - x0)
                        fde = ps_fde.tile([P, 8 * D], F32, tag="fde", space="PSUM")
                        for j in range(xc):
                            gg = x0 + j
                            rep = reps[gg // 4]
                            for wo in range(int(wlo[gg]), int(whi[gg]) + 1):
                                qt = qp.tile([P, P], F16, tag="qt")
                                nc.vector.tensor_tensor(
                                    out=qt[:],
                                    in0=ic_ext[:, wo:wo + 1].to_broadcast([P, P]),
                                    in1=rep[:, (gg % 4) * P:(gg % 4 + 1) * P],
                                    op=Alu.is_equal)
                                nc.tensor.matmul(
                                    fde[:, j * D:(j + 1) * D], lhsT=qt[:],
                                    rhs=fd_t[:, wo * D:(wo + 1) * D],
                                    start=(wo == int(wlo[gg])),
                                    stop=(wo == int(whi[gg])))
                        # x = fs + fde ; leaky ; e ; z
                        x_t = sb.tile([P, 8 * D], F16, tag="x")
                        nc.vector.tensor_tensor(
                            out=x_t[:, :xc * D],
                            in0=fsg[:, x0 * D:(x0 + xc) * D],
                            in1=fde[:, :xc * D], op=Alu.add)
                        xs = sb.tile([P, 8 * D], F16, tag="xs")
                        nc.vector.tensor_scalar_mul(
                            xs[:, :xc * D], x_t[:, :xc * D], GAT_SLOPE)
                        xl = sb.tile([P, 8 * D], F16, tag="xl")
                        nc.vector.tensor_tensor(
                            out=xl[:, :xc * D], in0=x_t[:, :xc * D],
                            in1=xs[:, :xc * D], op=Alu.max)
                        xa = sb.tile([P, 8 * D], F32, tag="xa")
                        nc.vector.tensor_tensor(
                            out=xa[:, :xc * D], in0=xl[:, :xc * D],
                            in1=a_l.rearrange("p (g d) -> p g d", g=1)
                            .to_broadcast([P, xc, D]),
                            op=Alu.mult)
                        e8 = sb.tile([P, 8], F32, tag="e8")
                        nc.vector.reduce_sum(
                            out=e8[:, :xc],
                            in_=xa[:, :xc * D].rearrange("p (g d) -> p g d", d=D),
                            axis=mybir.AxisListType.X)
                        z8 = sb.tile([P, 8], F32, tag="z8")
                        nc.scalar.activation(z8[:, :xc], e8[:, :xc], Act.Exp)
                        nc.vector.tensor_tensor(
                            out=w_big[:, x0 * (D + 1):(x0 + xc) * (D + 1)]
                            .rearrange("p (g d) -> p g d", d=D + 1)[:, :, 0:D],
                            in0=fsg[:, x0 * D:(x0 + xc) * D]
                            .rearrange("p (g d) -> p g d", d=D),
                            in1=z8[:, :xc].rearrange("p (g d) -> p g d", d=1)
                            .to_broadcast([P, xc, D]),
                            op=Alu.mult)
                        nc.vector.tensor_copy(
                            out=w_big[:, x0 * (D + 1):(x0 + xc) * (D + 1)]
                            .rearrange("p (g d) -> p g d", d=D + 1)[:, :, D:D + 1],
                            in_=z8[:, :xc].rearrange("p (g d) -> p g d", d=1))

                    # accumulation: window-major; each sub-tile contributes to
                    # every window in its [wlo, whi] range via shifted masks
                    for wo in range(wbi):
                        subs = [gg for gg in range(G)
                                if int(wlo[gg]) <= wo <= int(whi[gg])]
                        for si, gg in enumerate(subs):
                            q_t = qp.tile([P, P], F16, tag="q")
                            nc.vector.tensor_tensor(
                                out=q_t[:],
                                in0=dlc_t[:, gg:gg + 1].to_broadcast([P, P]),
                                in1=im_ext[:, wo * P:(wo + 1) * P],
                                op=Alu.is_equal)
                            nc.tensor.matmul(
                                acc[:, wo * (D + 1):(wo + 1) * (D + 1)],
                                lhsT=q_t[:],
                                rhs=w_big[:, gg * (D + 1):(gg + 1) * (D + 1)],
                                start=(si == 0),
                                stop=(si == len(subs) - 1))
                    # divide + store
                    out_t = sb.tile([P, g.WB * D], F32, tag="out")
                    for wo in range(wbi):
                        den = sb.tile([P, 1], F32, tag="den")
                        nc.vector.tensor_scalar_max(
                            den[:], acc[:, wo * (D + 1) + D:(wo + 1) * (D + 1)],
                            1e-30)
                        rec = sb.tile([P, 1], F32, tag="rec")
                        nc.vector.reciprocal(rec[:], den[:])
                        if resid_tab is None:
                            nc.vector.tensor_scalar_mul(
                                out_t[:, wo * D:(wo + 1) * D],
                                acc[:, wo * (D + 1):wo * (D + 1) + D],
                                rec[:, :1])
                        else:
                            tmp = sb.tile([P, D], F32, tag="dtmp")
                            nc.vector.tensor_scalar_mul(
                                tmp[:], acc[:, wo * (D + 1):wo * (D + 1) + D],
                                rec[:, :1])
                            nc.vector.tensor_tensor(
                                out=out_t[:, wo * D:(wo + 1) * D],
                                in0=tmp[:], in1=rs_t[:, wo * D:(wo + 1) * D],
                                op=Alu.add)
                    nc.sync.dma_start(
                        out=out_tensor.ap()[w_base * P:(w_base + wbi) * P, :]
                        .rearrange("(g p) d -> p g d", p=P),
                        in_=out_t[:, :wbi * D].rearrange("p (g d) -> p g d", d=D))

                    w_base += wbi
                    g_base += G

        # ------------------------------------------------------------------
        def epilogue_phase(l, octx=None, mm_bufs=2):
            with ExitStack() as _own:
                ctx = octx if octx is not None else _own
                sb = ctx.enter_context(tc.tile_pool(name=f"ep{l}", bufs=2))
                pst = ctx.enter_context(
                    tc.tile_pool(name=f"ept{l}", bufs=2, space="PSUM"))
                psm = ctx.enter_context(
                    tc.tile_pool(name=f"epm{l}", bufs=mm_bufs, space="PSUM"))
                BT = 8
                w1_l = w1_sb[:, l * 2 * D:(l + 1) * 2 * D]
                b1_l = b1_sb[:, l * 2 * D:(l + 1) * 2 * D]
                w2_l = w2_sb[:, l * 2 * D:(l + 1) * 2 * D]
                b2_l = b2_sb[:, l * 2:(l + 1) * 2]
                for t0 in range(0, UT, BT):
                    bt = min(BT, UT - t0)
                    rows = slice(t0 * P, (t0 + bt) * P)
                    ut = sb.tile([P, BT * D], F32, tag="eu")
                    nc.sync.dma_start(
                        out=ut[:, :bt * D].rearrange("p (g d) -> p g d", d=D),
                        in_=u_shards[l].ap()[rows, :].rearrange("(g p) d -> p g d", p=P))
                    pt = sb.tile([P, BT * D], F32, tag="epp")
                    nc.sync.dma_start(
                        out=pt[:, :bt * D].rearrange("p (g d) -> p g d", d=D),
                        in_=p_sh.ap()[rows, :].rearrange("(g p) d -> p g d", p=P))
                    qt_ = sb.tile([P, BT * D], F32, tag="epq")
                    nc.sync.dma_start(
                        out=qt_[:, :bt * D].rearrange("p (g d) -> p g d", d=D),
                        in_=q_sh.ap()[rows, :].rearrange("(g p) d -> p g d", p=P))
                    ot = sb.tile([P, BT * D], F32, tag="eo")
                    # per-k: transposes + matmuls into batched psum rows
                    mm_inf = psm.tile([P, BT * D], F32, tag="emmi")
                    mm_int = psm.tile([P, BT * D], F32, tag="emmj")
                    for k in range(bt):
                        ts = []
                        for srcp in (ut, pt, qt_):
                            tp = pst.tile([D, P], F32, tag="etp")
                            nc.tensor.transpose(
                                out=tp[:], in_=srcp[:, k * D:(k + 1) * D],
                                identity=ident[:])
                            ts.append(tp)
                        ct_inf = sb.tile([2 * D, P], F32, tag="ecti")
                        nc.vector.tensor_copy(out=ct_inf[0:D, :], in_=ts[0][:])
                        nc.vector.tensor_copy(out=ct_inf[D:2 * D, :], in_=ts[1][:])
                        ct_int = sb.tile([2 * D, P], F32, tag="ectj")
                        nc.vector.tensor_copy(out=ct_int[0:D, :], in_=ts[0][:])
                        nc.vector.tensor_copy(out=ct_int[D:2 * D, :], in_=ts[2][:])
                        nc.tensor.matmul(
                            mm_inf[:, k * D:(k + 1) * D], lhsT=ct_inf[:],
                            rhs=w1_l[:, 0:D], start=True, stop=True)
                        nc.tensor.matmul(
                            mm_int[:, k * D:(k + 1) * D], lhsT=ct_int[:],
                            rhs=w1_l[:, D:2 * D], start=True, stop=True)
                    # batched gating math across the bt tiles
                    sv2 = []
                    for mm_b, col in ((mm_inf, 0), (mm_int, 1)):
                        s1 = sb.tile([P, BT * D], F32, tag="es1")
                        nc.vector.tensor_tensor(
                            out=s1[:, :bt * D].rearrange("p (g d) -> p g d", d=D),
                            in0=mm_b[:, :bt * D].rearrange("p (g d) -> p g d", d=D),
                            in1=b1_l[:, col * D:(col + 1) * D]
                            .rearrange("p (g d) -> p g d", g=1)
                            .to_broadcast([P, bt, D]), op=Alu.add)
                        s1s = sb.tile([P, BT * D], F32, tag="es1s")
                        nc.vector.tensor_scalar_mul(
                            s1s[:, :bt * D], s1[:, :bt * D], MLP_SLOPE)
                        s1l = sb.tile([P, BT * D], F32, tag="es1l")
                        nc.vector.tensor_tensor(
                            out=s1l[:, :bt * D], in0=s1[:, :bt * D],
                            in1=s1s[:, :bt * D], op=Alu.max)
                        xw = sb.tile([P, BT * D], F32, tag="exw")
                        nc.vector.tensor_tensor(
                            out=xw[:, :bt * D].rearrange("p (g d) -> p g d", d=D),
                            in0=s1l[:, :bt * D].rearrange("p (g d) -> p g d", d=D),
                            in1=w2_l[:, col * D:(col + 1) * D]
                            .rearrange("p (g d) -> p g d", g=1)
                            .to_broadcast([P, bt, D]), op=Alu.mult)
                        sv0 = sb.tile([P, BT], F32, tag="esv0")
                        nc.vector.reduce_sum(
                            out=sv0[:, :bt],
                            in_=xw[:, :bt * D].rearrange("p (g d) -> p g d", d=D),
                            axis=mybir.AxisListType.X)
                        sv = sb.tile([P, BT], F32, tag="esv")
                        nc.vector.tensor_tensor(
                            out=sv[:, :bt].rearrange("p (g d) -> p g d", d=1),
                            in0=sv0[:, :bt].rearrange("p (g d) -> p g d", d=1),
                            in1=b2_l[:, col:col + 1]
                            .rearrange("p (g d) -> p g d", g=1)
                            .to_broadcast([P, bt, 1]), op=Alu.add)
                        svs = sb.tile([P, BT], F32, tag="esvs")
                        nc.vector.tensor_scalar_mul(
                            svs[:, :bt], sv[:, :bt], MLP_SLOPE)
                        svl = sb.tile([P, BT], F32, tag="esvl")
                        nc.vector.tensor_tensor(
                            out=svl[:, :bt], in0=sv[:, :bt],
                            in1=svs[:, :bt], op=Alu.max)
                        sv2.append(svl)
                    dg = sb.tile([P, BT], F32, tag="edg")
                    nc.vector.tensor_tensor(
                        out=dg[:, :bt], in0=sv2[0][:, :bt], in1=sv2[1][:, :bt],
                        op=Alu.subtract)
                    g0 = sb.tile([P, BT], F32, tag="eg0")
                    nc.scalar.activation(g0[:, :bt], dg[:, :bt], Act.Sigmoid)
                    pq = sb.tile([P, BT * D], F32, tag="epq2")
                    nc.vector.tensor_tensor(
                        out=pq[:, :bt * D], in0=pt[:, :bt * D],
                        in1=qt_[:, :bt * D], op=Alu.subtract)
                    gpq = sb.tile([P, BT * D], F32, tag="egpq")
                    nc.vector.tensor_tensor(
                        out=gpq[:, :bt * D].rearrange("p (g d) -> p g d", d=D),
                        in0=pq[:, :bt * D].rearrange("p (g d) -> p g d", d=D),
                        in1=g0[:, :bt].rearrange("p (g d) -> p g d", d=1)
                        .to_broadcast([P, bt, D]), op=Alu.mult)
                    uq = sb.tile([P, BT * D], F32, tag="euq")
                    nc.vector.tensor_tensor(
                        out=uq[:, :bt * D], in0=ut[:, :bt * D],
                        in1=qt_[:, :bt * D], op=Alu.add)
                    nc.vector.tensor_tensor(
                        out=ot[:, :bt * D], in0=uq[:, :bt * D],
                        in1=gpq[:, :bt * D], op=Alu.add)
                    nc.sync.dma_start(
                        out=u_shards[l + 1].ap()[rows, :]
                        .rearrange("(g p) d -> p g d", p=P),
                        in_=ot[:, :bt * D].rearrange("p (g d) -> p g d", d=D))

        # ------------------------------------------------------------------
        def hu_build_phase():
            """Assemble hu [UPAD, PD] fp16 (full) and hi_sh [IS, PD] (local)."""
            with ExitStack() as ctx:
                sb = ctx.enter_context(tc.tile_pool(name="hub", bufs=2))
                BT = 16
                for tabs, out_tab, n_tiles in ((u_tabs, hu_t, UPAD // P),
                                               (it_shards, hi_sh, IS // P)):
                    for t0 in range(0, n_tiles, BT):
                        bt = min(BT, n_tiles - t0)
                        rows = slice(t0 * P, (t0 + bt) * P)
                        big = sb.tile([P, BT * PD], F16, tag="hbig")
                        nc.vector.memset(big[:], 0)
                        for li, tab in enumerate(tabs):
                            ld = sb.tile([P, BT * D], F32, tag="hld")
                            nc.sync.dma_start(
                                out=ld[:, :bt * D].rearrange("p (g d) -> p g d", d=D),
                                in_=tab.ap()[rows, :]
                                .rearrange("(g p) d -> p g d", p=P))
                            nc.vector.tensor_copy(
                                out=big[:, :bt * PD]
                                .rearrange("p (g d) -> p g d", d=PD)
                                [:, :, li * D:(li + 1) * D],
                                in_=ld[:, :bt * D]
                                .rearrange("p (g d) -> p g d", d=D))
                        nc.sync.dma_start(
                            out=out_tab.ap()[rows, :]
                            .rearrange("(g p) d -> p g d", p=P),
                            in_=big[:, :bt * PD].rearrange("p (g d) -> p g d", d=PD))

        # ------------------------------------------------------------------
        def pred_phase():
            """Pred edges dst(item)-window grouped: gather hu[src], expand
            hi[dst] via one-hot matmuls, fused dot products."""
            g = pred
            gi = g_in[g.name]
            GM = g.G_max
            with ExitStack() as ctx:
                sb = ctx.enter_context(tc.tile_pool(name="pred", bufs=2))
                qp = ctx.enter_context(tc.tile_pool(name="predq", bufs=3))
                ps_rep = ctx.enter_context(
                    tc.tile_pool(name="predr", bufs=2, space="PSUM"))
                ps_hi = ctx.enter_context(
                    tc.tile_pool(name="predh", bufs=2, space="PSUM"))
                nb = g.nb
                w_base = 0
                g_base = 0
                call = 0
                for bi, wbi in enumerate(g.blocks):
                    G = int(g.G_blk[bi])
                    idx_t = sb.tile([P, GM * P // 16], I16, tag="idx")
                    c0 = g_base * P // 16
                    nc.sync.dma_start(
                        out=idx_t[:, :G * P // 16],
                        in_=gi["idx"].ap()[:, c0:c0 + G * P // 16])
                    dlr_t = sb.tile([1, GM * P], F16, tag="dlr")
                    nc.sync.dma_start(
                        out=dlr_t[:, :G * P],
                        in_=gi["dlr"].ap()[:, g_base * P:(g_base + G) * P])
                    hi_win = sb.tile([P, g.WB * PD], F16, tag="hiw")
                    nc.sync.dma_start(
                        out=hi_win[:, :wbi * PD],
                        in_=hi_sh.ap()[w_base * P:(w_base + wbi) * P, :]
                        .rearrange("(g p) d -> p g d", p=P))

                    hu_g = sb.tile([P, GM * PD], F16, tag="hug")
                    scol = 0
                    sg = 0
                    for b in range(nb):
                        ngb = int(g.seg[bi, b])
                        if ngb == 0:
                            continue
                        nidx = ngb * P
                        hi_row = min(hu_t.ap().shape[0], (b + 1) * BANK)
                        nc.gpsimd.dma_gather(
                            hu_g[:, sg * PD:(sg + ngb) * PD]
                            .rearrange("p (g d) -> p g d", d=PD),
                            hu_t.ap()[b * BANK:hi_row, :],
                            idx_t[:, scol:scol + nidx // 16],
                            nidx, nidx, PD, single_packet=False)
                        scol += nidx // 16
                        sg += ngb

                    wlo = g.sub_wlo[g_base:g_base + G]
                    whi = g.sub_whi[g_base:g_base + G]
                    reps = []
                    for r0 in range(0, G, 4):
                        rc = min(4, G - r0)
                        rep = ps_rep.tile([P, 4 * P], F32, tag="rep", space="PSUM")
                        nc.tensor.matmul(
                            rep[:, :rc * P], lhsT=onr[:],
                            rhs=dlr_t[:1, r0 * P:(r0 + rc) * P],
                            start=True, stop=True)
                        reps.append(rep)

                    dt_ = sb.tile([P, GM], F32, tag="pdot")
                    for x0 in range(0, G, 4):
                        xc = min(4, G - x0)
                        hi_exp = ps_hi.tile([P, 4 * PD], F32, tag="hie",
                                            space="PSUM")
                        for j in range(xc):
                            gg = x0 + j
                            rep = reps[gg // 4]
                            for wo in range(int(wlo[gg]), int(whi[gg]) + 1):
                                qt = qp.tile([P, P], F16, tag="qt")
                                nc.vector.tensor_tensor(
                                    out=qt[:],
                                    in0=ic_ext[:, wo:wo + 1].to_broadcast([P, P]),
                                    in1=rep[:, (gg % 4) * P:(gg % 4 + 1) * P],
                                    op=Alu.is_equal)
                                nc.tensor.matmul(
                                    hi_exp[:, j * PD:(j + 1) * PD], lhsT=qt[:],
                                    rhs=hi_win[:, wo * PD:(wo + 1) * PD],
                                    start=(wo == int(wlo[gg])),
                                    stop=(wo == int(whi[gg])))
                        prod = sb.tile([P, 4 * PD], F32, tag="pprod")
                        nc.vector.tensor_tensor(
                            out=prod[:, :xc * PD],
                            in0=hu_g[:, x0 * PD:(x0 + xc) * PD],
                            in1=hi_exp[:, :xc * PD], op=Alu.mult)
                        nc.vector.reduce_sum(
                            out=dt_[:, x0:x0 + xc],
                            in_=prod[:, :xc * PD]
                            .rearrange("p (g d) -> p g d", d=PD),
                            axis=mybir.AxisListType.X)
                    nc.sync.dma_start(
                        out=pred_out.ap()[:, g_base:g_base + G],
                        in_=dt_[:, :G])
                    w_base += wbi
                    g_base += G
                    call = (bi + 1) * nb

        # ------------------------------------------------------------------
        phase_order = ["null"]
        for l in range(L):
            phase_order += [f"proj{l}", f"rate{l}", f"rb{l}", f"tr{l}",
                            f"epi{l}", f"ag{l}"]
        phase_order += ["hu", "pred"]

        def run_until():
            import os as _os4
            kpair = _os4.environ.get("KPAIR", "1") == "1"
            if kphase == "null":
                return
            for l in range(L):
                proj_phase(l)
                if kphase == f"proj{l}":
                    return
                if kpair:
                    # [rb || tr] then [rate || epi]: compute of one phase
                    # overlaps the other's gather stream (single DMA queue
                    # is the only hard serial resource)
                    with ExitStack() as s1:
                        gat_phase(l, rb, fs_tab[("rb", l)][1],
                                  fd_shard[("rb", l)], q_sh, None,
                                  octx=s1, rep_bufs=1, fde_bufs=1)
                        gat_phase(l, tr, fs_tab[("tr", l)][1],
                                  fd_shard[("tr", l)], p_sh, None,
                                  octx=s1, rep_bufs=1, fde_bufs=1)
                    if kphase == f"tr{l}":
                        return
                    with ExitStack() as s2:
                        gat_phase(l, rate, fs_tab[("rate", l)][1],
                                  fd_shard[("rate", l)], it_shards[l + 1],
                                  it_shards[l],
                                  octx=s2, rep_bufs=1, fde_bufs=1)
                        epilogue_phase(l, octx=s2, mm_bufs=1)
                    if kphase == f"epi{l}":
                        return
                else:
                    gat_phase(l, rate, fs_tab[("rate", l)][1],
                              fd_shard[("rate", l)], it_shards[l + 1],
                              it_shards[l])
                    if kphase == f"rate{l}":
                        return
                    gat_phase(l, rb, fs_tab[("rb", l)][1],
                              fd_shard[("rb", l)], q_sh, None)
                    gat_phase(l, tr, fs_tab[("tr", l)][1],
                              fd_shard[("tr", l)], p_sh, None)
                    if kphase == f"tr{l}":
                        return
                    epilogue_phase(l)
                    if kphase == f"epi{l}":
                        return
                if _os4.environ.get("KNOAG") != "1":
                    for ai, ao in ((u_shards[l + 1], u_tabs[l + 1]),
                                   (it_shards[l + 1], it_tabs[l + 1])):
                        nc.gpsimd.collective_compute(
                            "AllGather", Alu.bypass, replica_groups=rg,
                            ins=[ai.ap()[:, :]], outs=[ao.ap()[:, :]])
                if kphase == f"ag{l}":
                    return
            hu_build_phase()
            if kphase == "hu":
                return
            pred_phase()

        run_until()
        if dbg_out is not None:
            dbg_tensors = dict(
                q_sh=q_sh, p_sh=p_sh, hu=hu_t, hi_sh=hi_sh,
                **{f"u_shard{i}": t for i, t in enumerate(u_shards)},
                **{f"it_shard{i}": t for i, t in enumerate(it_shards)},
                **{f"u_tab{i}": t for i, t in enumerate(u_tabs)},
                **{f"it_tab{i}": t for i, t in enumerate(it_tabs)},
                **{f"fs_{nm}{l}": fs_tab[(nm, l)][1] for nm in ("rate", "rb", "tr")
                   for l in range(L)},
                **{f"fsin_{nm}{l}": fs_tab[(nm, l)][0] for nm in ("rate", "rb", "tr")
                   for l in range(L)},
                **{f"fd_{nm}{l}": fd_shard[(nm, l)] for nm in ("rate", "rb", "tr")
                   for l in range(L)},
            )
            src_t = dbg_tensors[dbg_spec[0]]
            sdt = src_t.ap().dtype
            with ExitStack() as ctx:
                sbd = ctx.enter_context(tc.tile_pool(name="dbg", bufs=2))
                rows, cols = dbg_spec[1], dbg_spec[2]
                for r0 in range(0, rows, P):
                    rc = min(P, rows - r0)
                    t_ = sbd.tile([P, cols], sdt, tag="dbg")
                    nc.sync.dma_start(out=t_[:rc, :],
                                      in_=src_t.ap()[r0:r0 + rc, :])
                    if sdt != F32:
                        t2 = sbd.tile([P, cols], F32, tag="dbg2")
                        nc.vector.tensor_copy(out=t2[:rc, :], in_=t_[:rc, :])
                        t_ = t2
                    nc.sync.dma_start(out=dbg_out.ap()[r0:r0 + rc, :],
                                      in_=t_[:rc, :])

    nc.compile()
    return nc


# ---------------------------------------------------------------------------
# entry point
# ---------------------------------------------------------------------------

def _pad_rows(a, rows):
    out = np.zeros((rows, a.shape[1]), dtype=a.dtype)
    out[:a.shape[0]] = a
    return out


def prepare(inputs):
    """Host preprocessing: returns (hp, in_maps, pred)."""
    U, D = inputs["user_emb"].shape
    I = inputs["item_emb"].shape[0]
    L = inputs["rate_Ws"].shape[0]
    UT = _ceil(_ceil(U, P), N_CORES)
    IT = _ceil(_ceil(I, P), N_CORES)
    US, IS = UT * P, IT * P
    UPAD, IPAD = US * N_CORES, IS * N_CORES
    # gather elem size must be a multiple of 256 bytes -> PD*2 % 256 == 0
    PD = _ceil(D * (L + 1) * 2, 256) * 128

    rate_src = np.asarray(inputs["rate_src"])
    rate_dst = np.asarray(inputs["rate_dst"])
    trust_src = np.asarray(inputs["trust_src"])
    trust_dst = np.asarray(inputs["trust_dst"])

    rate = GatStruct("rate", rate_src, rate_dst, UPAD, IT, wb_cap=112)
    rb = GatStruct("rb", rate_dst, rate_src, IPAD, UT, wb_cap=72)
    tr = GatStruct("tr", trust_src, trust_dst, UPAD, UT, wb_cap=72)

    pos_src = np.asarray(inputs["pos_src"])
    pos_dst = np.asarray(inputs["pos_dst"])
    neg_src = np.asarray(inputs["neg_src"])
    neg_dst = np.asarray(inputs["neg_dst"])
    psrc = np.concatenate([pos_src, neg_src])
    pdst = np.concatenate([pos_dst, neg_dst])
    pred = GatStruct("pred", psrc, pdst, UPAD, IT, wb_cap=40,
                     want_slotmap=True)

    hp = dict(U=U, I=I, D=D, L=L, UT=UT, IT=IT, PD=PD,
              rate=rate, rb=rb, tr=tr, pred=pred)
    print(f"[kernel] struct: rate G={rate.G_total} WB={rate.WB} "
          f"blocks={len(rate.blocks)}; rb G={rb.G_total} WB={rb.WB}; "
          f"tr G={tr.G_total} WB={tr.WB}; pred G={pred.G_total} WB={pred.WB}")

    f16 = NPF16
    ue_pad = _pad_rows(inputs["user_emb"].astype(np.float32), UPAD)
    ie_pad = _pad_rows(inputs["item_emb"].astype(np.float32), IPAD)
    wu = np.concatenate([
        np.concatenate([inputs["rate_Ws"][l], inputs["tr_Ws"][l],
                        inputs["rb_Wd"][l], inputs["tr_Wd"][l]], axis=1)
        for l in range(L)], axis=1).astype(np.float32)
    bu = np.concatenate([
        np.tile(np.concatenate([inputs["rate_bs"][l], inputs["tr_bs"][l],
                                inputs["rb_bd"][l], inputs["tr_bd"][l]])[None, :],
                (P, 1))
        for l in range(L)], axis=1).astype(np.float32)
    wi = np.concatenate([
        np.concatenate([inputs["rate_Wd"][l], inputs["rb_Ws"][l]], axis=1)
        for l in range(L)], axis=1).astype(np.float32)
    bi_ = np.concatenate([
        np.tile(np.concatenate([inputs["rate_bd"][l], inputs["rb_bs"][l]])[None, :],
                (P, 1))
        for l in range(L)], axis=1).astype(np.float32)
    a_arrs = {}
    for nm in ("rate", "rb", "tr"):
        a_arrs[nm] = np.concatenate([
            np.tile(np.asarray(inputs[f"{nm}_a"][l])[None, :], (P, 1))
            for l in range(L)], axis=1).astype(np.float32)
    w1 = np.concatenate([
        np.concatenate([inputs["inf_W1"][l], inputs["int_W1"][l]], axis=1)
        for l in range(L)], axis=1).astype(np.float32)
    b1 = np.concatenate([
        np.tile(np.concatenate([inputs["inf_b1"][l], inputs["int_b1"][l]])[None, :],
                (P, 1))
        for l in range(L)], axis=1).astype(np.float32)
    w2 = np.concatenate([
        np.tile(np.concatenate([inputs["inf_W2"][l][:, 0],
                                inputs["int_W2"][l][:, 0]])[None, :], (P, 1))
        for l in range(L)], axis=1).astype(np.float32)
    b2 = np.concatenate([
        np.tile(np.array([[inputs["inf_b2"][l][0], inputs["int_b2"][l][0]]],
                         dtype=np.float32), (P, 1))
        for l in range(L)], axis=1).astype(np.float32)
    iota = np.arange(P, dtype=np.float32)
    im_ext = np.concatenate(
        [np.tile(iota[None, :] + 128 * w, (P, 1)) for w in range(8)],
        axis=1).astype(f16)
    ic_ext = np.concatenate(
        [iota[:, None] + 128 * w for w in range(8)], axis=1).astype(f16)

    wblob = np.zeros((P, 2692), dtype=np.float32)
    wblob[:64, 0:512] = wu
    wblob[:, 512:1024] = bu
    wblob[:64, 1024:1280] = wi
    wblob[:, 1280:1536] = bi_
    wblob[:, 1536:1664] = a_arrs["rate"]
    wblob[:, 1664:1792] = a_arrs["rb"]
    wblob[:, 1792:1920] = a_arrs["tr"]
    wblob[:, 1920:2176] = w1
    wblob[:, 2176:2432] = b1
    wblob[:, 2432:2688] = w2
    wblob[:, 2688:2692] = b2
    fblob = np.zeros((P, 1160), dtype=f16)
    fblob[:, 0:1024] = im_ext
    fblob[:, 1024:1032] = ic_ext
    fblob[0, 1032:1160] = 1.0

    in_maps = []
    for c in range(N_CORES):
        m = {
            "user_emb": ue_pad, "item_emb": ie_pad,
            "u_shard0": ue_pad[c * US:(c + 1) * US],
            "it_shard0": ie_pad[c * IS:(c + 1) * IS],
            "wblob": wblob, "fblob": fblob,
        }
        for g in (rate, rb, tr, pred):
            m[f"{g.name}_idx"] = g.idx16[c]
            m[f"{g.name}_dlc"] = g.dlc[c]
            m[f"{g.name}_dlr"] = g.dlr[c]
        in_maps.append(m)
    return hp, in_maps, pred, psrc


def kernel(**inputs):
    import os
    import time as _t
    hp, in_maps, pred, psrc = prepare(inputs)

    kdbg = os.environ.get("KDBG")
    if kdbg:
        U, D = inputs["user_emb"].shape
        L = inputs["rate_Ws"].shape[0]
        UT, IT = hp["UT"], hp["IT"]
        US, IS = UT * P, IT * P
        UPAD, IPAD = US * N_CORES, IS * N_CORES
        PD = hp["PD"]
        shp = {}
        for i in range(L + 1):
            shp[f"u_shard{i}"] = (US, D); shp[f"it_shard{i}"] = (IS, D)
            shp[f"u_tab{i}"] = (UPAD, D); shp[f"it_tab{i}"] = (IPAD, D)
        for l in range(L):
            shp[f"fs_rate{l}"] = (UPAD, D); shp[f"fs_tr{l}"] = (UPAD, D)
            shp[f"fs_rb{l}"] = (IPAD, D)
            shp[f"fsin_rate{l}"] = (US, D); shp[f"fsin_tr{l}"] = (US, D)
            shp[f"fsin_rb{l}"] = (IS, D)
            shp[f"fd_rate{l}"] = (IS, D); shp[f"fd_rb{l}"] = (US, D)
            shp[f"fd_tr{l}"] = (US, D)
        shp["q_sh"] = (US, D); shp["p_sh"] = (US, D)
        shp["hu"] = (UPAD, PD); shp["hi"] = (IPAD, PD)
        hp["dbg_spec"] = (kdbg, *shp[kdbg])

    t_b = _t.time()
    nc = build_program(hp)
    print(f"[kernel] build+compile: {_t.time() - t_b:.1f}s")

    trace = os.environ.get("KTRACE") == "1"
    t_run = _t.time()
    res = run_bass_kernel_spmd(nc, in_maps, core_ids=list(range(N_CORES)),
                               trace=trace)
    print(f"[kernel] device run wall: {_t.time() - t_run:.1f}s")
    global LAST_RES, LAST_HP, LAST_EXEC_NS
    LAST_RES, LAST_HP, LAST_EXEC_NS = res, hp, res.exec_time_ns
    if os.environ.get("KBENCH") == "1":
        tmin = bench_pjrt(nc, in_maps, iters=int(os.environ.get("KBENCH_ITERS", "4")))
        LAST_EXEC_NS = int(tmin * 1e9)

    # ---- assemble outputs ----
    E = len(psrc)
    n_pos = len(np.asarray(inputs["pos_src"]))
    out = np.zeros((E,), dtype=np.float32)
    for c in range(N_CORES):
        vals = res.results[c]["pred_out"]  # [128, G_total]
        smap = pred.slotmap[c]
        gidx = np.arange(len(smap))
        v = vals[gidx % P, gidx // P]
        ok = smap >= 0
        out[smap[ok]] = v[ok]
    pos = out[:n_pos].reshape(-1, 1)
    neg = out[n_pos:].reshape(-1, 1)
    return pos, neg
